# revision 1
# baseline (speedup 1.0000x reference)
"""Trainium2 Bass kernel for DeformBottleneckBlock (DCNv2 bottleneck).

Sharding: 8 cores = (batch b in 0..3) x (H-half in 0..1); each core computes
output rows [lo, lo+50) of one image. Fully data-parallel, no collectives.

Per-core pipeline:
  A) conv1 1x1 (fp32, bn1 folded, bias via indicator channel so out-of-image
     rows are exactly zero) -> out1 channel-major bf16, padded strip
     [128, 2, 58*108]; then PE transposes build a shingled token-major buffer
     tm[x_pad, y, 512ch] (own 256 ch + right neighbor 256 ch, 1KB/token).
  B) offset conv 3x3 (im2col shifted views, PSUM-accumulated) -> om [27,5120].
     Field pipeline on packed [36,1280] tiles (partition = 4k+chunk):
     bilinear corner weight maps W00..11 (validity-masked, mask-folded) and
     int16 gather indices (floor via round(x+7.5) cast).
  C) per (k, 512-chunk): two dma_gathers (corner row pairs, 4 SWDGE queues),
     rank-1 PE broadcast of the 4 weight maps, 4 muls + 3 adds -> s_k bf16,
     36 PSUM-accumulated matmuls (w2, bn2 folded) -> bn2 bias + relu -> out2.
  D) conv3 1x1 (bf16) + residual add (fp32 x) + bn3 bias + relu -> out.
"""

import numpy as np
import ml_dtypes

B, CIN, H, W = 4, 1024, 100, 100
CB, COUT, KOFF = 256, 1024, 27

PAD = 4
RSTRIP = 58
WPAD = 108
NPOS = 5120
NCHUNK = 512
NCHUNKS = NPOS // NCHUNK
NW = 1280
NVALID = 5000


def _build_program():
    import concourse.bacc as bacc
    import concourse.mybir as mybir
    from concourse.tile import TileContext
    from concourse.bass import ts
    from concourse.masks import make_identity

    dt = mybir.dt
    AF = mybir.ActivationFunctionType
    ALU = mybir.AluOpType
    f32, bf16, i16, i32 = dt.float32, dt.bfloat16, dt.int16, dt.int32

    nc = bacc.Bacc("TRN2", target_bir_lowering=False, num_swdge_queues=4)

    xs_d = nc.dram_tensor("xs", [8, 128, RSTRIP * W], f32, kind="ExternalInput")
    ind_d = nc.dram_tensor("ind", [1, RSTRIP * W], f32, kind="ExternalInput")
    w1T_d = nc.dram_tensor("w1T", [8, 128, CB], f32, kind="ExternalInput")
    w1b_d = nc.dram_tensor("w1b", [1, CB], f32, kind="ExternalInput")
    woffT_d = nc.dram_tensor("woffT", [9, 2, 128, KOFF], bf16, kind="ExternalInput")
    boff_d = nc.dram_tensor("boff", [KOFF, 1], f32, kind="ExternalInput")
    w2T_d = nc.dram_tensor("w2T", [9, 2, 128, CB], bf16, kind="ExternalInput")
    b2_d = nc.dram_tensor("b2", [128, 2], f32, kind="ExternalInput")
    w3T_d = nc.dram_tensor("w3T", [2, 128, COUT], bf16, kind="ExternalInput")
    b3_d = nc.dram_tensor("b3", [128, 8], f32, kind="ExternalInput")
    basey_d = nc.dram_tensor("basey", [36, NW], f32, kind="ExternalInput")
    basex_d = nc.dram_tensor("basex", [36, NW], f32, kind="ExternalInput")
    kia_d = nc.dram_tensor("kia", [36, 1], f32, kind="ExternalInput")
    kja_d = nc.dram_tensor("kja", [36, 1], f32, kind="ExternalInput")
    vb_d = nc.dram_tensor("vb", [36, 4], f32, kind="ExternalInput")
    sel_d = nc.dram_tensor("sel", [36, 36 * 128], bf16, kind="ExternalInput")
    idxstg_t_d = nc.dram_tensor("idxstg_t", [1, 9 * NPOS], i16)
    idxstg_b_d = nc.dram_tensor("idxstg_b", [1, 9 * NPOS], i16)
    out_d = nc.dram_tensor("out", [8, 128, NVALID], f32, kind="ExternalOutput")

    with TileContext(nc) as tc:
        with tc.tile_pool(name="persist", bufs=1) as pp, \
             tc.tile_pool(name="io", bufs=2) as iop:

            tm = pp.tile([128, RSTRIP, 4, 128], bf16)
            w00 = pp.tile([36, NW], bf16)
            w01 = pp.tile([36, NW], bf16)
            w10 = pp.tile([36, NW], bf16)
            w11 = pp.tile([36, NW], bf16)
            idx_top = pp.tile([128, 9 * 320], i16)
            idx_bot = pp.tile([128, 9 * 320], i16)
            b2 = pp.tile([128, 2], f32)
            nc.sync.dma_start(out=b2, in_=b2_d[:, :])
            b3 = pp.tile([128, 8], f32)
            nc.sync.dma_start(out=b3, in_=b3_d[:, :])
            sel = pp.tile([36, 36 * 128], bf16)
            nc.sync.dma_start(out=sel, in_=sel_d[:, :])

            nc.vector.memset(tm[:, :, :, :], 0)

            with tc.tile_pool(name="omscope", bufs=1) as omp:
                om = omp.tile([KOFF, NPOS], f32)
                nc.vector.memset(om[:, :], 0)

                # ======== Stage A/B: conv1, tm build, offset conv ========
                with tc.tile_pool(name="stageab", bufs=1) as ap, \
                     tc.tile_pool(name="xck", bufs=2) as xp, \
                     tc.tile_pool(name="psA", bufs=2, space="PSUM") as psA:

                    out1_cm = ap.tile([128, 2, RSTRIP * WPAD], bf16)
                    nc.vector.memset(out1_cm[:, :, :], 0)
                    w1T = ap.tile([128, 8, CB], f32)
                    for kt in range(8):
                        nc.sync.dma_start(out=w1T[:, kt, :], in_=w1T_d[kt, :, :])
                    w1b = ap.tile([1, CB], f32)
                    nc.sync.dma_start(out=w1b, in_=w1b_d[:, :])
                    woffT = ap.tile([128, 9, 2, KOFF], bf16)
                    for tap in range(9):
                        for ct in range(2):
                            nc.sync.dma_start(out=woffT[:, tap, ct, :],
                                              in_=woffT_d[tap, ct, :, :])
                    boff = ap.tile([KOFF, 1], f32)
                    nc.sync.dma_start(out=boff, in_=boff_d[:, :])
                    ident = ap.tile([128, 128], bf16)
                    make_identity(nc, ident)

                    cmv = out1_cm.rearrange("p c (r w) -> p c r w", w=WPAD)

                    chunks = [(4 * i, 4) for i in range(14)] + [(56, 2)]
                    for (r0, nrows) in chunks:
                        npos = nrows * W
                        xt = xp.tile([128, 8, 4 * W], f32, tag="xchunk")
                        for kt in range(8):
                            nc.sync.dma_start(out=xt[:, kt, :npos],
                                              in_=xs_d[kt, :, r0 * W:r0 * W + npos])
                        indt = xp.tile([1, 4 * W], f32, tag="indchunk")
                        nc.sync.dma_start(out=indt[:, :npos],
                                          in_=ind_d[:, r0 * W:r0 * W + npos])
                        for mt in range(2):
                            ps = psA.tile([128, 4 * W], f32, tag="convps")
                            for kt in range(8):
                                nc.tensor.matmul(ps[:, :npos], w1T[:, kt, ts(mt, 128)],
                                                 xt[:, kt, :npos],
                                                 start=(kt == 0), stop=False)
                            nc.tensor.matmul(ps[:, :npos], w1b[:, ts(mt, 128)],
                                             indt[:, :npos], start=False, stop=True)
                            nc.scalar.activation(
                                cmv[:, mt, r0:r0 + nrows, PAD:PAD + W],
                                ps[:, :npos].rearrange("p (r w) -> p r w", w=W),
                                AF.Relu)

                    for y in range(RSTRIP):
                        for ct in range(2):
                            for sh in range(2):
                                ncols = WPAD if sh == 0 else WPAD - 1
                                pst = psA.tile([128, 128], bf16, tag="tpose")
                                nc.tensor.transpose(pst[:ncols, :],
                                                    cmv[:, ct, y, sh:sh + ncols],
                                                    ident)
                                nc.scalar.copy(tm[:ncols, y, 2 * sh + ct, :],
                                               pst[:ncols, :])

                    for rc in range(10):
                        r0 = rc * 5
                        npos = 5 * W
                        ps = psA.tile([KOFF, 5 * W], f32, tag="omps")
                        first = True
                        for tap in range(9):
                            ti, tj = divmod(tap, 3)
                            rhs = cmv[:, :, r0 + 3 + ti:r0 + 3 + ti + 5,
                                      PAD + tj - 1:PAD + tj - 1 + W]
                            for ct in range(2):
                                nc.tensor.matmul(
                                    ps.rearrange("p (r w) -> p r w", w=W),
                                    woffT[:, tap, ct, :], rhs[:, ct],
                                    start=first, stop=(tap == 8 and ct == 1))
                                first = False
                        nc.scalar.activation(om[:, rc * npos:(rc + 1) * npos], ps,
                                             AF.Identity, bias=boff[:, :])

                # ======== Stage B2: packed field pipeline ========
                with tc.tile_pool(name="fieldsc", bufs=1) as fc:
                    _tc_n = [0]

                    def T(tag, d=f32):
                        _tc_n[0] += 1
                        return fc.tile([36, NW], d, tag=tag,
                                       name=f"fld_{tag}_{_tc_n[0]}")

                    dyp = T("pA")
                    dxp = T("pB")
                    mrp = T("pC")
                    basey = T("pD")
                    basex = T("pE")
                    for k in range(9):
                        nc.sync.dma_start(
                            out=dyp[4 * k:4 * k + 4, :],
                            in_=om[2 * k:2 * k + 1, :].rearrange(
                                "q (c n) -> q c n", n=NW))
                        nc.sync.dma_start(
                            out=dxp[4 * k:4 * k + 4, :],
                            in_=om[2 * k + 1:2 * k + 2, :].rearrange(
                                "q (c n) -> q c n", n=NW))
                        nc.sync.dma_start(
                            out=mrp[4 * k:4 * k + 4, :],
                            in_=om[18 + k:19 + k, :].rearrange(
                                "q (c n) -> q c n", n=NW))
                    nc.sync.dma_start(out=basey, in_=basey_d[:, :])
                    nc.sync.dma_start(out=basex, in_=basex_d[:, :])
                    kia = fc.tile([36, 1], f32)
                    nc.sync.dma_start(out=kia, in_=kia_d[:, :])
                    kja = fc.tile([36, 1], f32)
                    nc.sync.dma_start(out=kja, in_=kja_d[:, :])
                    vb = fc.tile([36, 4], f32)
                    nc.sync.dma_start(out=vb, in_=vb_d[:, :])

                    ayy = T("pF")
                    nc.vector.tensor_add(ayy, dyp, basey)          # pA,pD free
                    nc.scalar.activation(ayy, ayy, AF.Identity, bias=kia[:, :])
                    ayi = T("pA", i32)
                    nc.vector.tensor_copy(ayi, ayy)                # floor(yy)+8
                    ayf = T("pD")
                    nc.vector.tensor_copy(ayf, ayi)                # pA free
                    wyh = T("pG")                                  # wy - 0.5
                    nc.vector.tensor_sub(wyh, ayy, ayf)            # pF free
                    msig = T("pF")
                    nc.scalar.activation(msig, mrp, AF.Sigmoid)    # pC free
                    bxx = T("pC")
                    nc.vector.tensor_add(bxx, dxp, basex)          # pB,pE free
                    nc.scalar.activation(bxx, bxx, AF.Identity, bias=kja[:, :])
                    bxi = T("pB", i32)
                    nc.vector.tensor_copy(bxi, bxx)
                    bxf = T("pE")
                    nc.vector.tensor_copy(bxf, bxi)                # pB free
                    wxh = T("pH")
                    nc.vector.tensor_sub(wxh, bxx, bxf)            # pC free

                    # gather indices from floors (ayf, bxf live)
                    idxpf = T("pB")
                    nc.vector.tensor_scalar(idxpf, ayf, 128.0, -1032.0,
                                            ALU.mult, ALU.add)
                    nc.vector.tensor_add(idxpf, idxpf, bxf)
                    idx_pt = fc.tile([36, NW], i16, tag="pI1")
                    nc.vector.tensor_copy(idx_pt, idxpf)
                    nc.vector.tensor_scalar(idxpf, idxpf, 128.0, None, ALU.add)
                    idx_pb = fc.tile([36, NW], i16, tag="pI2")
                    nc.vector.tensor_copy(idx_pb, idxpf)           # pB free
                    # relayout packed -> wrapped+replicated via DRAM bounce
                    nc.sync.dma_start(
                        out=idxstg_t_d[0, :].rearrange("(r n) -> r n", n=NW),
                        in_=idx_pt[:, :])
                    nc.sync.dma_start(
                        out=idxstg_b_d[0, :].rearrange("(r n) -> r n", n=NW),
                        in_=idx_pb[:, :])
                    for g in range(8):
                        nc.sync.dma_start(
                            out=idx_top[16 * g:16 * g + 16, :].rearrange(
                                "p (k s) -> p k s", k=9),
                            in_=idxstg_t_d[0, :].rearrange(
                                "(k s q) -> q k s", k=9, q=16))
                        nc.sync.dma_start(
                            out=idx_bot[16 * g:16 * g + 16, :].rearrange(
                                "p (k s) -> p k s", k=9),
                            in_=idxstg_b_d[0, :].rearrange(
                                "(k s q) -> q k s", k=9, q=16))

                    # validity + weight maps
                    def cmp_range(dst, src, lo_ap, hi_ap, tmp):
                        nc.vector.tensor_scalar(tmp, src, lo_ap, None, ALU.is_ge)
                        nc.vector.tensor_scalar(dst, src, hi_ap, None, ALU.is_le)
                        nc.vector.tensor_mul(dst, dst, tmp)

                    tmp = T("pA")
                    vy0 = T("pB")
                    cmp_range(vy0, ayf, vb[:, 0:1], vb[:, 1:2], tmp)
                    vy1 = T("pC")
                    cmp_range(vy1, ayf, vb[:, 2:3], vb[:, 3:4], tmp)   # pD free
                    atop = T("pD")
                    nc.vector.tensor_scalar(atop, wyh, -1.0, 0.5, ALU.mult, ALU.add)
                    nc.vector.tensor_mul(atop, atop, msig)
                    nc.vector.tensor_mul(atop, atop, vy0)              # pB free
                    abot = T("pB")
                    nc.vector.tensor_scalar(abot, wyh, 0.5, None, ALU.add)
                    nc.vector.tensor_mul(abot, abot, msig)
                    nc.vector.tensor_mul(abot, abot, vy1)   # pG, pF, pC free
                    vx0 = T("pC")
                    cmp_range(vx0, bxf, 12.0, 111.0, tmp)
                    vx1 = T("pF")
                    cmp_range(vx1, bxf, 11.0, 110.0, tmp)              # pE free
                    c0 = T("pE")
                    nc.vector.tensor_scalar(c0, wxh, -1.0, 0.5, ALU.mult, ALU.add)
                    nc.vector.tensor_mul(c0, c0, vx0)                  # pC free
                    c1 = T("pC")
                    nc.vector.tensor_scalar(c1, wxh, 0.5, None, ALU.add)
                    nc.vector.tensor_mul(c1, c1, vx1)                  # pH, pF free
                    nc.vector.tensor_mul(w00, atop, c0)
                    nc.vector.tensor_mul(w01, atop, c1)
                    nc.vector.tensor_mul(w10, abot, c0)
                    nc.vector.tensor_mul(w11, abot, c1)

            # ======== Stage C + D ========
            with tc.tile_pool(name="stagecd", bufs=1) as cp, \
                 tc.tile_pool(name="gath", bufs=2) as gp, \
                 tc.tile_pool(name="psC", bufs=1, space="PSUM") as psC, \
                 tc.tile_pool(name="psW", bufs=4, space="PSUM") as psW:

                out2 = cp.tile([128, 2, NPOS], bf16)
                w2T = cp.tile([128, 9, 2, CB], bf16)
                for k in range(9):
                    for ct in range(2):
                        nc.sync.dma_start(out=w2T[:, k, ct, :], in_=w2T_d[k, ct, :, :])

                tmflat = tm.rearrange("p r a b -> p (r a b)")
                wmaps = (w00, w01, w10, w11)
                for ch in range(NCHUNKS):
                    accs = [psC.tile([128, NCHUNK], f32, tag=f"dacc{mt}",
                                     name=f"dacc_{ch}_{mt}")
                            for mt in range(2)]
                    for k in range(9):
                        gt = gp.tile([128, 4, NCHUNK], bf16, tag="gtop")
                        gb = gp.tile([128, 4, NCHUNK], bf16, tag="gbot")
                        isl_t = idx_top[:, k * 320 + ch * 32:k * 320 + (ch + 1) * 32]
                        isl_b = idx_bot[:, k * 320 + ch * 32:k * 320 + (ch + 1) * 32]
                        nc.gpsimd.dma_gather(
                            out_ap=gt[:, :, :], in_ap=tmflat, idxs_ap=isl_t,
                            num_idxs=NCHUNK, num_idxs_reg=NCHUNK,
                            elem_size=512, transpose=True,
                            sbuf_tokens_per_rank=128, sbuf_free_dim_per_rank=1024,
                            queue_num=(2 * k) % 4)
                        nc.gpsimd.dma_gather(
                            out_ap=gb[:, :, :], in_ap=tmflat, idxs_ap=isl_b,
                            num_idxs=NCHUNK, num_idxs_reg=NCHUNK,
                            elem_size=512, transpose=True,
                            sbuf_tokens_per_rank=128, sbuf_free_dim_per_rank=1024,
                            queue_num=(2 * k + 1) % 4)
                        j0 = ch * NCHUNK
                        c_a, o_a = divmod(j0, NW)
                        n_a = min(NCHUNK, NW - o_a)
                        ra = 4 * k + c_a
                        sel_a = sel[:, ra * 128:(ra + 1) * 128]
                        sel_b = (sel[:, (ra + 1) * 128:(ra + 2) * 128]
                                 if n_a < NCHUNK else None)
                        wrep = []
                        for ci, wmap in enumerate(wmaps):
                            pw = psW.tile([128, NCHUNK], f32, tag="wps",
                                          name=f"wps_{ch}_{k}_{ci}")
                            nc.tensor.matmul(pw[:, :n_a], sel_a,
                                             wmap[:, o_a:o_a + n_a],
                                             start=True, stop=True)
                            if n_a < NCHUNK:
                                nc.tensor.matmul(pw[:, n_a:], sel_b,
                                                 wmap[:, 0:NCHUNK - n_a],
                                                 start=True, stop=True)
                            wsb = gp.tile([128, NCHUNK], bf16, tag=f"wsb{ci}",
                                          name=f"wsb_{ch}_{k}_{ci}")
                            nc.scalar.copy(wsb, pw)
                            wrep.append(wsb)
                        s0 = gp.tile([128, 2, NCHUNK], bf16, tag="s0")
                        s1 = gp.tile([128, 2, NCHUNK], bf16, tag="s1")

                        def bc(wsb):
                            return wsb.unsqueeze(1).broadcast_to([128, 2, NCHUNK])

                        nc.vector.tensor_mul(s0, gt[:, 0:2, :], bc(wrep[0]))
                        nc.vector.tensor_mul(s1, gt[:, 2:4, :], bc(wrep[1]))
                        nc.vector.tensor_add(s0, s0, s1)
                        nc.vector.tensor_mul(s1, gb[:, 0:2, :], bc(wrep[2]))
                        nc.vector.tensor_add(s0, s0, s1)
                        nc.vector.tensor_mul(s1, gb[:, 2:4, :], bc(wrep[3]))
                        nc.vector.tensor_add(s0, s0, s1)
                        for mt in range(2):
                            for ct in range(2):
                                nc.tensor.matmul(accs[mt], w2T[:, k, ct, ts(mt, 128)],
                                                 s0[:, ct, :],
                                                 start=(k == 0 and ct == 0),
                                                 stop=(k == 8 and ct == 1))
                    for mt in range(2):
                        nc.scalar.activation(out2[:, mt, ch * NCHUNK:(ch + 1) * NCHUNK],
                                             accs[mt], AF.Relu, bias=b2[:, mt:mt + 1])

                # ---- Stage D ----
                w3T = cp.tile([128, 2, COUT], bf16)
                for ct in range(2):
                    nc.sync.dma_start(out=w3T[:, ct, :], in_=w3T_d[ct, :, :])
                for ch in range(NCHUNKS):
                    n0 = ch * NCHUNK
                    nn = min(NCHUNK, NVALID - n0)
                    if nn <= 0:
                        break
                    for mt in range(8):
                        xr = iop.tile([128, NCHUNK], f32, tag="xres")
                        nc.sync.dma_start(out=xr[:, :nn],
                                          in_=xs_d[mt, :, PAD * W + n0:PAD * W + n0 + nn])
                        ps = psC.tile([128, NCHUNK], f32, tag="c3ps", bufs=2, name=f"c3ps_{ch}_{mt}")
                        for ct in range(2):
                            nc.tensor.matmul(ps[:, :nn], w3T[:, ct, ts(mt, 128)],
                                             out2[:, ct, n0:n0 + nn],
                                             start=(ct == 0), stop=(ct == 1))
                        t = iop.tile([128, NCHUNK], f32, tag="resid")
                        nc.vector.tensor_add(t[:, :nn], ps[:, :nn], xr[:, :nn])
                        o = iop.tile([128, NCHUNK], f32, tag="obuf")
                        nc.scalar.activation(o[:, :nn], t[:, :nn], AF.Relu,
                                             bias=b3[:, mt:mt + 1])
                        nc.sync.dma_start(out=out_d[mt, :, n0:n0 + nn], in_=o[:, :nn])

    nc.finalize()
    return nc


_NC_CACHE = None


def _get_nc():
    global _NC_CACHE
    if _NC_CACHE is None:
        _NC_CACHE = _build_program()
    return _NC_CACHE


def _prep_inputs(x, w1, s1, b1, w_off, b_off, w2, s2, b2, w3, s3, b3):
    bf16 = ml_dtypes.bfloat16
    f32 = np.float32
    x = np.asarray(x, f32)
    w1f = np.asarray(w1, f32) * np.asarray(s1, f32)[:, None]
    w1T = np.ascontiguousarray(w1f.T.reshape(8, 128, CB))
    w1b = np.ascontiguousarray(np.asarray(b1, f32)[None, :])
    woffT = np.zeros((9, 2, 128, KOFF), bf16)
    w_off = np.asarray(w_off, f32)
    for tap in range(9):
        ti, tj = divmod(tap, 3)
        wt = w_off[:, :, ti, tj]
        for ct in range(2):
            woffT[tap, ct] = wt[:, ct * 128:(ct + 1) * 128].T.astype(bf16)
    boff = np.asarray(b_off, f32)[:, None]
    w2f = np.asarray(w2, f32) * np.asarray(s2, f32)[:, None, None, None]
    w2T = np.zeros((9, 2, 128, CB), bf16)
    for k in range(9):
        ki, kj = divmod(k, 3)
        wk = w2f[:, :, ki, kj]
        for ct in range(2):
            w2T[k, ct] = wk[:, ct * 128:(ct + 1) * 128].T.astype(bf16)
    b2t = np.ascontiguousarray(np.asarray(b2, f32).reshape(2, 128).T)
    w3f = np.asarray(w3, f32) * np.asarray(s3, f32)[:, None]
    w3T = np.zeros((2, 128, COUT), bf16)
    for ct in range(2):
        w3T[ct] = w3f[:, ct * 128:(ct + 1) * 128].T.astype(bf16)
    b3t = np.ascontiguousarray(np.asarray(b3, f32).reshape(8, 128).T)

    j = np.arange(NPOS)
    y_loc = np.where(j < NVALID, PAD + j // W, 20).astype(f32)
    x_pad = np.where(j < NVALID, PAD + j % W, 50).astype(f32)
    basey = np.zeros((36, NW), f32)
    basex = np.zeros((36, NW), f32)
    kia = np.zeros((36, 1), f32)
    kja = np.zeros((36, 1), f32)
    for k in range(9):
        ki, kj = divmod(k, 3)
        for c in range(4):
            basey[4 * k + c] = y_loc[c * NW:(c + 1) * NW]
            basex[4 * k + c] = x_pad[c * NW:(c + 1) * NW]
            kia[4 * k + c] = ki - 1 + 7.5
            kja[4 * k + c] = kj - 1 + 7.5
    selm = np.zeros((36, 36 * 128), bf16)
    for r in range(36):
        selm[r, r * 128:(r + 1) * 128] = 1.0

    shared = dict(w1T=w1T, w1b=w1b, woffT=np.asarray(woffT), boff=boff,
                  w2T=np.asarray(w2T), b2=b2t, w3T=np.asarray(w3T), b3=b3t,
                  basey=basey, basex=basex, kia=kia, kja=kja,
                  sel=np.asarray(selm))

    in_maps = []
    for core in range(8):
        b, half = core // 2, core % 2
        lo = half * 50
        xs = np.zeros((CIN, RSTRIP, W), f32)
        vlo = max(0, lo - PAD)
        vhi = min(H - 1, lo + 49 + PAD)
        loc0 = vlo - (lo - PAD)
        nrows = vhi - vlo + 1
        xs[:, loc0:loc0 + nrows, :] = x[b, :, vlo:vhi + 1, :]
        indv = np.zeros((RSTRIP, W), f32)
        indv[loc0:loc0 + nrows, :] = 1.0
        vbm = np.zeros((36, 4), f32)
        vbm[:, 0] = loc0 + 8
        vbm[:, 1] = loc0 + nrows - 1 + 8
        vbm[:, 2] = loc0 + 8 - 1
        vbm[:, 3] = loc0 + nrows - 1 + 8 - 1
        in_maps.append(dict(shared,
                            xs=np.ascontiguousarray(xs.reshape(8, 128, RSTRIP * W)),
                            ind=indv.reshape(1, -1), vb=vbm))
    return in_maps


def kernel(**inputs):
    from concourse.bass_utils import run_bass_kernel_spmd
    nc = _get_nc()
    in_maps = _prep_inputs(**inputs)
    res = run_bass_kernel_spmd(nc, in_maps, core_ids=list(range(8)))
    out = np.zeros((B, COUT, H, W), np.float32)
    for core in range(8):
        b, half = core // 2, core % 2
        lo = half * 50
        o = res.results[core]["out"].reshape(COUT, 50, W)
        out[b, :, lo:lo + 50, :] = o
    return out



# revision 18
# speedup vs baseline: 2.1487x; 2.1487x over previous
"""Trainium2 Bass kernel for DeformBottleneckBlock (DCNv2 bottleneck).

Sharding: 8 cores = (batch b in 0..3) x (H-half in 0..1); each core computes
output rows [lo, lo+50) of one image. Fully data-parallel, no collectives.

Per-core pipeline:
  A) conv1 1x1 (fp32, bn1 folded, bias via indicator channel so out-of-image
     rows are exactly zero) -> out1 channel-major bf16, padded strip
     [128, 2, 58*108]; then PE transposes build a shingled token-major buffer
     tm[x_pad, y, 512ch] (own 256 ch + right neighbor 256 ch, 1KB/token).
  B) offset conv 3x3 (im2col shifted views, PSUM-accumulated) -> om [27,5120].
     Field pipeline on packed [36,1280] tiles (partition = 4k+chunk):
     bilinear corner weight maps W00..11 (validity-masked, mask-folded) and
     int16 gather indices (floor via round(x+7.5) cast).
  C) per (k, 512-chunk): two dma_gathers (corner row pairs, 4 SWDGE queues),
     rank-1 PE broadcast of the 4 weight maps, 4 muls + 3 adds -> s_k bf16,
     36 PSUM-accumulated matmuls (w2, bn2 folded) -> bn2 bias + relu -> out2.
  D) conv3 1x1 (bf16) + residual add (fp32 x) + bn3 bias + relu -> out.
"""

import numpy as np
import ml_dtypes

B, CIN, H, W = 4, 1024, 100, 100
CB, COUT, KOFF = 256, 1024, 27

PAD = 4
RSTRIP = 58
WPAD = 108
NPOS = 5120
NCHUNK = 512
NCHUNKS = NPOS // NCHUNK
NW = 1280
NVALID = 5000


def _build_program():
    import concourse.bacc as bacc
    import concourse.mybir as mybir
    from concourse.tile import TileContext
    from concourse.bass import ts
    from concourse.masks import make_identity

    dt = mybir.dt
    AF = mybir.ActivationFunctionType
    ALU = mybir.AluOpType
    f32, bf16, i16, i32 = dt.float32, dt.bfloat16, dt.int16, dt.int32

    nc = bacc.Bacc("TRN2", target_bir_lowering=False, num_swdge_queues=4)

    xs_d = nc.dram_tensor("xs", [8, 128, RSTRIP * W], f32, kind="ExternalInput")
    ind_d = nc.dram_tensor("ind", [1, RSTRIP * W], f32, kind="ExternalInput")
    w1T_d = nc.dram_tensor("w1T", [8, 128, CB], f32, kind="ExternalInput")
    w1b_d = nc.dram_tensor("w1b", [1, CB], f32, kind="ExternalInput")
    woffT_d = nc.dram_tensor("woffT", [9, 2, 128, KOFF], bf16, kind="ExternalInput")
    boff_d = nc.dram_tensor("boff", [KOFF, 1], f32, kind="ExternalInput")
    w2T_d = nc.dram_tensor("w2T", [9, 2, 128, CB], bf16, kind="ExternalInput")
    b2_d = nc.dram_tensor("b2", [128, 2], f32, kind="ExternalInput")
    w3T_d = nc.dram_tensor("w3T", [2, 128, COUT], bf16, kind="ExternalInput")
    b3_d = nc.dram_tensor("b3", [128, 8], f32, kind="ExternalInput")
    basey_d = nc.dram_tensor("basey", [36, NW], f32, kind="ExternalInput")
    basex_d = nc.dram_tensor("basex", [36, NW], f32, kind="ExternalInput")
    kia_d = nc.dram_tensor("kia", [36, 1], f32, kind="ExternalInput")
    kja_d = nc.dram_tensor("kja", [36, 1], f32, kind="ExternalInput")
    vb_d = nc.dram_tensor("vb", [36, 4], f32, kind="ExternalInput")
    sel_d = nc.dram_tensor("sel", [36, 36 * 128], bf16, kind="ExternalInput")
    # idx staging: per tap padded to 48*128; xbar-transposed to wrap-128 layout
    dstg_t_d = nc.dram_tensor("dstg_t", [9, 48 * 128], i16)
    dstg_b_d = nc.dram_tensor("dstg_b", [9, 48 * 128], i16)
    rrep_t_d = nc.dram_tensor("rrep_t", [128, 432], i16)
    rrep_b_d = nc.dram_tensor("rrep_b", [128, 432], i16)
    out_d = nc.dram_tensor("out", [8, 128, NVALID], f32, kind="ExternalOutput")
    import os
    dbg = os.environ.get("KERNEL_DEBUG", "0") == "1"
    if dbg:
        dbg_idx_t = nc.dram_tensor("dbg_idx_t", [128, 8 * 9 * 48], i16,
                                   kind="ExternalOutput")
        dbg_wsb = nc.dram_tensor("dbg_wsb", [128, 4 * 640], bf16,
                                 kind="ExternalOutput")
        dbg_out2 = nc.dram_tensor("dbg_out2", [128, 2 * NPOS], bf16,
                                  kind="ExternalOutput")
        dbg_s0 = nc.dram_tensor("dbg_s0", [128, 2 * 640], bf16,
                                kind="ExternalOutput")
        dbg_ipt = nc.dram_tensor("dbg_ipt", [36, NW], i16,
                                 kind="ExternalOutput")
        dbg_om = nc.dram_tensor("dbg_om", [KOFF, NPOS], f32,
                                kind="ExternalOutput")

    with TileContext(nc) as tc:
        with tc.tile_pool(name="persist", bufs=1) as pp, \
             tc.tile_pool(name="io", bufs=2) as iop:

            tm = pp.tile([128, RSTRIP, 4, 128], bf16)
            w00 = pp.tile([36, NW], bf16)
            w01 = pp.tile([36, NW], bf16)
            w10 = pp.tile([36, NW], bf16)
            w11 = pp.tile([36, NW], bf16)
            # wrap-16 gather idx, replicated x8: [128=(8 rep,16 q), m, k, t]
            idxall_t = pp.tile([128, 8, 9, 48], i16)
            idxall_b = pp.tile([128, 8, 9, 48], i16)
            b2 = pp.tile([128, 2], f32)
            nc.sync.dma_start(out=b2, in_=b2_d[:, :])
            b3 = pp.tile([128, 8], f32)
            nc.sync.dma_start(out=b3, in_=b3_d[:, :])
            sel = pp.tile([36, 36 * 128], bf16)
            nc.sync.dma_start(out=sel, in_=sel_d[:, :])

            nc.vector.memset(tm[:, :, :, :], 0)

            with tc.tile_pool(name="omscope", bufs=1) as omp:
                om = omp.tile([KOFF, NPOS], f32)
                nc.vector.memset(om[:, :], 0)

                # ======== Stage A/B: conv1, tm build, offset conv ========
                with tc.tile_pool(name="stageab", bufs=1) as ap, \
                     tc.tile_pool(name="xck", bufs=2) as xp, \
                     tc.tile_pool(name="psA", bufs=2, space="PSUM") as psA:

                    out1_cm = ap.tile([128, 2, RSTRIP * WPAD], bf16)
                    nc.vector.memset(out1_cm[:, :, :], 0)
                    w1T = ap.tile([128, 8, CB], f32)
                    for kt in range(8):
                        nc.sync.dma_start(out=w1T[:, kt, :], in_=w1T_d[kt, :, :])
                    w1b = ap.tile([1, CB], f32)
                    nc.sync.dma_start(out=w1b, in_=w1b_d[:, :])
                    woffT = ap.tile([128, 9, 2, KOFF], bf16)
                    for tap in range(9):
                        for ct in range(2):
                            nc.sync.dma_start(out=woffT[:, tap, ct, :],
                                              in_=woffT_d[tap, ct, :, :])
                    boff = ap.tile([KOFF, 1], f32)
                    nc.sync.dma_start(out=boff, in_=boff_d[:, :])
                    ident = ap.tile([128, 128], bf16)
                    make_identity(nc, ident)

                    cmv = out1_cm.rearrange("p c (r w) -> p c r w", w=WPAD)

                    chunks = [(4 * i, 4) for i in range(14)] + [(56, 2)]
                    for (r0, nrows) in chunks:
                        npos = nrows * W
                        xt = xp.tile([128, 8, 4 * W], f32, tag="xchunk")
                        for kt in range(8):
                            nc.sync.dma_start(out=xt[:, kt, :npos],
                                              in_=xs_d[kt, :, r0 * W:r0 * W + npos])
                        indt = xp.tile([1, 4 * W], f32, tag="indchunk")
                        nc.sync.dma_start(out=indt[:, :npos],
                                          in_=ind_d[:, r0 * W:r0 * W + npos])
                        for mt in range(2):
                            ps = psA.tile([128, 4 * W], f32, tag="convps")
                            for kt in range(8):
                                nc.tensor.matmul(ps[:, :npos], w1T[:, kt, ts(mt, 128)],
                                                 xt[:, kt, :npos],
                                                 start=(kt == 0), stop=False)
                            nc.tensor.matmul(ps[:, :npos], w1b[:, ts(mt, 128)],
                                             indt[:, :npos], start=False, stop=True)
                            nc.scalar.activation(
                                cmv[:, mt, r0:r0 + nrows, PAD:PAD + W],
                                ps[:, :npos].rearrange("p (r w) -> p r w", w=W),
                                AF.Relu)

                    for y in range(RSTRIP):
                        for ct in range(2):
                            for sh in range(2):
                                ncols = WPAD if sh == 0 else WPAD - 1
                                pst = psA.tile([128, 128], bf16, tag="tpose")
                                nc.tensor.transpose(pst[:ncols, :],
                                                    cmv[:, ct, y, sh:sh + ncols],
                                                    ident)
                                nc.scalar.copy(tm[:ncols, y, 2 * sh + ct, :],
                                               pst[:ncols, :])

                    for rc in range(10):
                        r0 = rc * 5
                        npos = 5 * W
                        ps = psA.tile([KOFF, 5 * W], f32, tag="omps")
                        first = True
                        for tap in range(9):
                            ti, tj = divmod(tap, 3)
                            rhs = cmv[:, :, r0 + 3 + ti:r0 + 3 + ti + 5,
                                      PAD + tj - 1:PAD + tj - 1 + W]
                            for ct in range(2):
                                nc.tensor.matmul(
                                    ps.rearrange("p (r w) -> p r w", w=W),
                                    woffT[:, tap, ct, :], rhs[:, ct],
                                    start=first, stop=(tap == 8 and ct == 1))
                                first = False
                        nc.scalar.activation(om[:, rc * npos:(rc + 1) * npos], ps,
                                             AF.Identity, bias=boff[:, :])
                    if dbg:
                        nc.sync.dma_start(out=dbg_om[:, :], in_=om[:, :])

                # ======== Stage B2: packed field pipeline ========
                with tc.tile_pool(name="fieldsc", bufs=1) as fc:
                    _tc_n = [0]

                    def T(tag, d=f32):
                        _tc_n[0] += 1
                        return fc.tile([36, NW], d, tag=tag,
                                       name=f"fld_{tag}_{_tc_n[0]}")

                    dyp = T("pA")
                    dxp = T("pB")
                    mrp = T("pC")
                    basey = T("pD")
                    basex = T("pE")
                    for k in range(9):
                        nc.sync.dma_start(
                            out=dyp[4 * k:4 * k + 4, :],
                            in_=om[2 * k:2 * k + 1, :].rearrange(
                                "q (c n) -> q c n", n=NW))
                        nc.sync.dma_start(
                            out=dxp[4 * k:4 * k + 4, :],
                            in_=om[2 * k + 1:2 * k + 2, :].rearrange(
                                "q (c n) -> q c n", n=NW))
                        nc.sync.dma_start(
                            out=mrp[4 * k:4 * k + 4, :],
                            in_=om[18 + k:19 + k, :].rearrange(
                                "q (c n) -> q c n", n=NW))
                    nc.sync.dma_start(out=basey, in_=basey_d[:, :])
                    nc.sync.dma_start(out=basex, in_=basex_d[:, :])
                    kia = fc.tile([36, 1], f32)
                    nc.sync.dma_start(out=kia, in_=kia_d[:, :])
                    kja = fc.tile([36, 1], f32)
                    nc.sync.dma_start(out=kja, in_=kja_d[:, :])
                    vb = fc.tile([36, 4], f32)
                    nc.sync.dma_start(out=vb, in_=vb_d[:, :])

                    ayy = T("pF")
                    nc.vector.tensor_add(ayy, dyp, basey)          # pA,pD free
                    nc.scalar.activation(ayy, ayy, AF.Identity, bias=kia[:, :])
                    ayi = T("pA", i32)
                    nc.vector.tensor_copy(ayi, ayy)                # floor(yy)+8
                    ayf = T("pD")
                    nc.vector.tensor_copy(ayf, ayi)                # pA free
                    wyh = T("pG")                                  # wy - 0.5
                    nc.vector.tensor_sub(wyh, ayy, ayf)            # pF free
                    msig = T("pF")
                    nc.scalar.activation(msig, mrp, AF.Sigmoid)    # pC free
                    bxx = T("pC")
                    nc.vector.tensor_add(bxx, dxp, basex)          # pB,pE free
                    nc.scalar.activation(bxx, bxx, AF.Identity, bias=kja[:, :])
                    bxi = T("pB", i32)
                    nc.vector.tensor_copy(bxi, bxx)
                    bxf = T("pE")
                    nc.vector.tensor_copy(bxf, bxi)                # pB free
                    wxh = T("pH")
                    nc.vector.tensor_sub(wxh, bxx, bxf)            # pC free

                    # gather indices from floors (ayf, bxf live)
                    idxpf = T("pB")
                    nc.vector.tensor_scalar(idxpf, ayf, 128.0, -1032.0,
                                            ALU.mult, ALU.add)
                    nc.vector.tensor_add(idxpf, idxpf, bxf)
                    idx_pt = fc.tile([36, NW], i16, tag="pI1")
                    nc.vector.tensor_copy(idx_pt, idxpf)
                    nc.vector.tensor_scalar(idxpf, idxpf, 128.0, None, ALU.add)
                    idx_pb = fc.tile([36, NW], i16, tag="pI2")
                    nc.vector.tensor_copy(idx_pb, idxpf)           # pB free
                    # relayout packed p-order -> wrap-128 via xbar DMA
                    # transpose, then replicate x8 via DRAM round trip.
                    if dbg:
                        nc.sync.dma_start(out=dbg_ipt[:, :], in_=idx_pt[:, :])
                    zpad = fc.tile([9, 1024], i16, tag="zpad")
                    nc.vector.memset(zpad, 0)
                    for wi, (idx_p, dstg_d, rrep_d, idxall) in enumerate((
                            (idx_pt, dstg_t_d, rrep_t_d, idxall_t),
                            (idx_pb, dstg_b_d, rrep_b_d, idxall_b))):
                        nc.sync.dma_start(out=dstg_d[:, NPOS:], in_=zpad[:, :])
                        for k in range(9):
                            nc.sync.dma_start(
                                out=dstg_d[k, :NPOS].rearrange(
                                    "(c n) -> c n", n=NW),
                                in_=idx_p[4 * k:4 * k + 4, :])
                        w128 = fc.tile([128, 432], i16, tag="w128",
                                       name=f"w128_{wi}")
                        nc.sync.dma_start(
                            out=w128[:, :],
                            in_=dstg_d.rearrange("k (r x) -> (k r) x", x=128),
                            transpose=True)
                        nc.sync.dma_start(out=rrep_d[:, :], in_=w128[:, :])
                        for g in range(8):
                            nc.sync.dma_start(
                                out=idxall[16 * g:16 * g + 16, :, :, :],
                                in_=rrep_d.rearrange(
                                    "(m q) (k t) -> q m k t", q=16, t=48))

                    # validity + weight maps
                    def cmp_range(dst, src, lo_ap, hi_ap, tmp):
                        nc.vector.tensor_scalar(tmp, src, lo_ap, None, ALU.is_ge)
                        nc.vector.tensor_scalar(dst, src, hi_ap, None, ALU.is_le)
                        nc.vector.tensor_mul(dst, dst, tmp)

                    tmp = T("pA")
                    vy0 = T("pB")
                    cmp_range(vy0, ayf, vb[:, 0:1], vb[:, 1:2], tmp)
                    vy1 = T("pC")
                    cmp_range(vy1, ayf, vb[:, 2:3], vb[:, 3:4], tmp)   # pD free
                    atop = T("pD")
                    nc.vector.tensor_scalar(atop, wyh, -1.0, 0.5, ALU.mult, ALU.add)
                    nc.vector.tensor_mul(atop, atop, msig)
                    nc.vector.tensor_mul(atop, atop, vy0)              # pB free
                    abot = T("pB")
                    nc.vector.tensor_scalar(abot, wyh, 0.5, None, ALU.add)
                    nc.vector.tensor_mul(abot, abot, msig)
                    nc.vector.tensor_mul(abot, abot, vy1)   # pG, pF, pC free
                    vx0 = T("pC")
                    cmp_range(vx0, bxf, 12.0, 111.0, tmp)
                    vx1 = T("pF")
                    cmp_range(vx1, bxf, 11.0, 110.0, tmp)              # pE free
                    c0 = T("pE")
                    nc.vector.tensor_scalar(c0, wxh, -1.0, 0.5, ALU.mult, ALU.add)
                    nc.vector.tensor_mul(c0, c0, vx0)                  # pC free
                    c1 = T("pC")
                    nc.vector.tensor_scalar(c1, wxh, 0.5, None, ALU.add)
                    nc.vector.tensor_mul(c1, c1, vx1)                  # pH, pF free
                    nc.vector.tensor_mul(w00, atop, c0)
                    nc.vector.tensor_mul(w01, atop, c1)
                    nc.vector.tensor_mul(w10, abot, c0)
                    nc.vector.tensor_mul(w11, abot, c1)

            # ======== Stage C + D ========
            # chunk m in 0..7 covers interleaved positions p = 128*s + 16*m + q
            # (s in 0..39, q in 0..15); gather slot i = 16*s + q.
            MCH = 640
            with tc.tile_pool(name="stagecd", bufs=1) as cp, \
                 tc.tile_pool(name="gath", bufs=2) as gp, \
                 tc.tile_pool(name="psC", bufs=1, space="PSUM") as psC, \
                 tc.tile_pool(name="psW", bufs=2, space="PSUM") as psW:

                out2 = cp.tile([128, 2, NPOS], bf16)
                o2v = out2.rearrange("p c (s x) -> p c s x", x=128)
                w2T = cp.tile([128, 9, 2, CB], bf16)
                for k in range(9):
                    for ct in range(2):
                        nc.sync.dma_start(out=w2T[:, k, ct, :], in_=w2T_d[k, ct, :, :])

                tmflat = tm.rearrange("p r a b -> p (r a b)")
                wmaps = (w00, w01, w10, w11)
                wvs = [w.rearrange("p (s x) -> p s x", x=128) for w in wmaps]
                gq = [0]  # SWDGE queue round-robin, aligned with sem lanes
                for m in range(8):
                    accA = [psC.tile([128, 512], f32, tag=f"daccA{mt}",
                                     name=f"daccA_{m}_{mt}") for mt in range(2)]
                    accB = [psC.tile([128, 128], f32, tag=f"daccB{mt}",
                                     name=f"daccB_{m}_{mt}") for mt in range(2)]
                    for k in range(9):
                        gt = gp.tile([128, 4, MCH], bf16, tag="gtop")
                        gb = gp.tile([128, 4, MCH], bf16, tag="gbot")
                        for (gx, idxall) in ((gt, idxall_t), (gb, idxall_b)):
                            nc.gpsimd.dma_gather(
                                out_ap=gx[:, :, :], in_ap=tmflat,
                                idxs_ap=idxall[:, m, k, 0:40],
                                num_idxs=MCH, num_idxs_reg=MCH,
                                elem_size=512, transpose=True,
                                sbuf_tokens_per_rank=128,
                                sbuf_free_dim_per_rank=1024,
                                queue_num=gq[0] % 4)
                            gq[0] += 1
                        wrep = []
                        for ci, wv in enumerate(wvs):
                            rhs = wv[:, :, 16 * m:16 * m + 16]
                            pwA = psW.tile([128, 512], f32, tag="wpsA",
                                           name=f"wpsA_{m}_{k}_{ci}")
                            pwB = psW.tile([128, 128], f32, tag="wpsB",
                                           name=f"wpsB_{m}_{k}_{ci}")
                            pAv = pwA.rearrange("p (s x) -> p s x", x=16)
                            for c in range(3):
                                nc.tensor.matmul(
                                    pAv[:, 10 * c:10 * c + 10, :],
                                    sel[:, (4 * k + c) * 128:(4 * k + c + 1) * 128],
                                    rhs, start=True, stop=True)
                            sel3 = sel[:, (4 * k + 3) * 128:(4 * k + 4) * 128]
                            nc.tensor.matmul(pAv[:, 30:32, :], sel3,
                                             rhs[:, 0:2, :], start=True, stop=True)
                            nc.tensor.matmul(
                                pwB.rearrange("p (s x) -> p s x", x=16),
                                sel3, rhs[:, 2:10, :], start=True, stop=True)
                            wsb = gp.tile([128, MCH], bf16, tag=f"wsb{ci}",
                                          name=f"wsb_{m}_{k}_{ci}")
                            nc.scalar.copy(wsb[:, 0:512], pwA)
                            nc.scalar.copy(wsb[:, 512:640], pwB)
                            wrep.append(wsb)
                        s0 = gp.tile([128, 2, MCH], bf16, tag="s0")
                        s1 = gp.tile([128, 2, MCH], bf16, tag="s1")

                        def bc(wsb):
                            return wsb.unsqueeze(1).broadcast_to([128, 2, MCH])

                        nc.vector.tensor_mul(s0, gt[:, 0:2, :], bc(wrep[0]))
                        nc.vector.tensor_mul(s1, gt[:, 2:4, :], bc(wrep[1]))
                        nc.vector.tensor_add(s0, s0, s1)
                        nc.vector.tensor_mul(s1, gb[:, 0:2, :], bc(wrep[2]))
                        nc.vector.tensor_add(s0, s0, s1)
                        nc.vector.tensor_mul(s1, gb[:, 2:4, :], bc(wrep[3]))
                        nc.vector.tensor_add(s0, s0, s1)
                        if dbg and m == 3 and k == 0:
                            for ci in range(4):
                                nc.sync.dma_start(
                                    out=dbg_wsb[:, 640 * ci:640 * (ci + 1)],
                                    in_=wrep[ci][:, :])
                            nc.sync.dma_start(
                                out=dbg_s0[:, :].rearrange(
                                    "p (c n) -> p c n", n=MCH),
                                in_=s0[:, :, :])
                        for mt in range(2):
                            for ct in range(2):
                                st = (k == 0 and ct == 0)
                                sp = (k == 8 and ct == 1)
                                w2s = w2T[:, k, ct, ts(mt, 128)]
                                nc.tensor.matmul(accA[mt], w2s, s0[:, ct, 0:512],
                                                 start=st, stop=sp)
                                nc.tensor.matmul(accB[mt], w2s, s0[:, ct, 512:640],
                                                 start=st, stop=sp)
                    for mt in range(2):
                        nc.scalar.activation(
                            o2v[:, mt, 0:32, 16 * m:16 * m + 16],
                            accA[mt].rearrange("p (s x) -> p s x", x=16),
                            AF.Relu, bias=b2[:, mt:mt + 1])
                        nc.scalar.activation(
                            o2v[:, mt, 32:40, 16 * m:16 * m + 16],
                            accB[mt].rearrange("p (s x) -> p s x", x=16),
                            AF.Relu, bias=b2[:, mt:mt + 1])

                if dbg:
                    nc.sync.dma_start(
                        out=dbg_idx_t[:, :].rearrange(
                            "p (m k t) -> p m k t", k=9, t=48),
                        in_=idxall_t[:, :, :, :])
                    nc.sync.dma_start(
                        out=dbg_out2[:, :].rearrange("p (c n) -> p c n", n=NPOS),
                        in_=out2[:, :, :])

                # ---- Stage D ----
                w3T = cp.tile([128, 2, COUT], bf16)
                for ct in range(2):
                    nc.sync.dma_start(out=w3T[:, ct, :], in_=w3T_d[ct, :, :])
                for ch in range(NCHUNKS):
                    n0 = ch * NCHUNK
                    nn = min(NCHUNK, NVALID - n0)
                    if nn <= 0:
                        break
                    for mt in range(8):
                        xr = iop.tile([128, NCHUNK], f32, tag="xres")
                        nc.sync.dma_start(out=xr[:, :nn],
                                          in_=xs_d[mt, :, PAD * W + n0:PAD * W + n0 + nn])
                        ps = psC.tile([128, NCHUNK], f32, tag=f"daccA{mt % 2}",
                                      name=f"c3ps_{ch}_{mt}")
                        for ct in range(2):
                            nc.tensor.matmul(ps[:, :nn], w3T[:, ct, ts(mt, 128)],
                                             out2[:, ct, n0:n0 + nn],
                                             start=(ct == 0), stop=(ct == 1))
                        t = iop.tile([128, NCHUNK], f32, tag="resid")
                        nc.vector.tensor_add(t[:, :nn], ps[:, :nn], xr[:, :nn])
                        o = iop.tile([128, NCHUNK], f32, tag="obuf")
                        nc.scalar.activation(o[:, :nn], t[:, :nn], AF.Relu,
                                             bias=b3[:, mt:mt + 1])
                        nc.sync.dma_start(out=out_d[mt, :, n0:n0 + nn], in_=o[:, :nn])

    nc.finalize()
    return nc


_NC_CACHE = None


def _get_nc():
    global _NC_CACHE
    if _NC_CACHE is None:
        _NC_CACHE = _build_program()
    return _NC_CACHE


def _prep_inputs(x, w1, s1, b1, w_off, b_off, w2, s2, b2, w3, s3, b3):
    bf16 = ml_dtypes.bfloat16
    f32 = np.float32
    x = np.asarray(x, f32)
    w1f = np.asarray(w1, f32) * np.asarray(s1, f32)[:, None]
    w1T = np.ascontiguousarray(w1f.T.reshape(8, 128, CB))
    w1b = np.ascontiguousarray(np.asarray(b1, f32)[None, :])
    woffT = np.zeros((9, 2, 128, KOFF), bf16)
    w_off = np.asarray(w_off, f32)
    for tap in range(9):
        ti, tj = divmod(tap, 3)
        wt = w_off[:, :, ti, tj]
        for ct in range(2):
            woffT[tap, ct] = wt[:, ct * 128:(ct + 1) * 128].T.astype(bf16)
    boff = np.asarray(b_off, f32)[:, None]
    w2f = np.asarray(w2, f32) * np.asarray(s2, f32)[:, None, None, None]
    w2T = np.zeros((9, 2, 128, CB), bf16)
    for k in range(9):
        ki, kj = divmod(k, 3)
        wk = w2f[:, :, ki, kj]
        for ct in range(2):
            w2T[k, ct] = wk[:, ct * 128:(ct + 1) * 128].T.astype(bf16)
    b2t = np.ascontiguousarray(np.asarray(b2, f32).reshape(2, 128).T)
    w3f = np.asarray(w3, f32) * np.asarray(s3, f32)[:, None]
    w3T = np.zeros((2, 128, COUT), bf16)
    for ct in range(2):
        w3T[ct] = w3f[:, ct * 128:(ct + 1) * 128].T.astype(bf16)
    b3t = np.ascontiguousarray(np.asarray(b3, f32).reshape(8, 128).T)

    j = np.arange(NPOS)
    y_loc = np.where(j < NVALID, PAD + j // W, 20).astype(f32)
    x_pad = np.where(j < NVALID, PAD + j % W, 50).astype(f32)
    basey = np.zeros((36, NW), f32)
    basex = np.zeros((36, NW), f32)
    kia = np.zeros((36, 1), f32)
    kja = np.zeros((36, 1), f32)
    for k in range(9):
        ki, kj = divmod(k, 3)
        for c in range(4):
            basey[4 * k + c] = y_loc[c * NW:(c + 1) * NW]
            basex[4 * k + c] = x_pad[c * NW:(c + 1) * NW]
            kia[4 * k + c] = ki - 1 + 7.5
            kja[4 * k + c] = kj - 1 + 7.5
    selm = np.zeros((36, 36 * 128), bf16)
    for r in range(36):
        selm[r, r * 128:(r + 1) * 128] = 1.0

    shared = dict(w1T=w1T, w1b=w1b, woffT=np.asarray(woffT), boff=boff,
                  w2T=np.asarray(w2T), b2=b2t, w3T=np.asarray(w3T), b3=b3t,
                  basey=basey, basex=basex, kia=kia, kja=kja,
                  sel=np.asarray(selm))

    in_maps = []
    for core in range(8):
        b, half = core // 2, core % 2
        lo = half * 50
        xs = np.zeros((CIN, RSTRIP, W), f32)
        vlo = max(0, lo - PAD)
        vhi = min(H - 1, lo + 49 + PAD)
        loc0 = vlo - (lo - PAD)
        nrows = vhi - vlo + 1
        xs[:, loc0:loc0 + nrows, :] = x[b, :, vlo:vhi + 1, :]
        indv = np.zeros((RSTRIP, W), f32)
        indv[loc0:loc0 + nrows, :] = 1.0
        vbm = np.zeros((36, 4), f32)
        vbm[:, 0] = loc0 + 8
        vbm[:, 1] = loc0 + nrows - 1 + 8
        vbm[:, 2] = loc0 + 8 - 1
        vbm[:, 3] = loc0 + nrows - 1 + 8 - 1
        in_maps.append(dict(shared,
                            xs=np.ascontiguousarray(xs.reshape(8, 128, RSTRIP * W)),
                            ind=indv.reshape(1, -1), vb=vbm))
    return in_maps


def kernel(**inputs):
    from concourse.bass_utils import run_bass_kernel_spmd
    nc = _get_nc()
    in_maps = _prep_inputs(**inputs)
    res = run_bass_kernel_spmd(nc, in_maps, core_ids=list(range(8)))
    out = np.zeros((B, COUT, H, W), np.float32)
    for core in range(8):
        b, half = core // 2, core % 2
        lo = half * 50
        o = res.results[core]["out"].reshape(COUT, 50, W)
        out[b, :, lo:lo + 50, :] = o
    return out



# revision 35
# speedup vs baseline: 3.1293x; 1.4564x over previous
"""Trainium2 Bass kernel for DeformBottleneckBlock (DCNv2 bottleneck).

Sharding: 8 cores = (batch b in 0..3) x (H-half in 0..1); each core computes
output rows [lo, lo+50) of one image. Fully data-parallel, no collectives.

Per-core pipeline:
  A) conv1 1x1 (fp32, bn1 folded, bias via indicator channel so out-of-image
     rows are exactly zero) -> out1 channel-major bf16, padded strip
     [128, 2, 58*108]; then PE transposes build a shingled token-major buffer
     tm[x_pad, y, 512ch] (own 256 ch + right neighbor 256 ch, 1KB/token).
  B) offset conv 3x3 (im2col shifted views, PSUM-accumulated) -> om [27,5120].
     Field pipeline on packed [36,1280] tiles (partition = 4k+chunk):
     bilinear corner weight maps W00..11 (validity-masked, mask-folded) and
     int16 gather indices (floor via round(x+7.5) cast).
     Idx relayout to the gather's wrap-16 format without tiny-descriptor
     DMAs: contiguous DRAM staging [9,48*128] -> one xbar DMA-transpose
     -> wrap-128 [128,432] -> DRAM round trip replicates x8 -> idxall.
  C) chunks are wrap-interleaved: chunk m (640 slots) covers positions
     p = 128*s + 16*m + q so idx tiles are contiguous idxall slices.
     Per (k,m): two dma_gathers (corner row pairs, 4 SWDGE queues,
     sem-lane-aligned round-robin), sel-matmul broadcast of the 4 weight
     maps via strided rhs views (4 row-blocks + tail), 4 muls + 3 adds ->
     s_k bf16, PSUM-accumulated w2 matmuls (512+128 col splits), bn2 bias
     + relu written back in p-order via strided activation dst views.
  D) conv3 1x1 (bf16) + residual add (bf16 x) + bn3 bias + relu -> out.
"""

import numpy as np
import ml_dtypes

B, CIN, H, W = 4, 1024, 100, 100
CB, COUT, KOFF = 256, 1024, 27

PAD = 4
RSTRIP = 58
WPAD = 108
NPOS = 5120
NCHUNK = 512
NCHUNKS = NPOS // NCHUNK
NW = 1280
NVALID = 5000


def _build_program():
    import concourse.bacc as bacc
    import concourse.mybir as mybir
    from concourse.tile import TileContext
    from concourse.bass import ts
    from concourse.masks import make_identity

    dt = mybir.dt
    AF = mybir.ActivationFunctionType
    ALU = mybir.AluOpType
    f32, bf16, i16, i32 = dt.float32, dt.bfloat16, dt.int16, dt.int32

    nc = bacc.Bacc("TRN2", target_bir_lowering=False, num_swdge_queues=4)

    xs_d = nc.dram_tensor("xs", [8, 128, RSTRIP * W], bf16, kind="ExternalInput")
    ind_d = nc.dram_tensor("ind", [1, RSTRIP * W], bf16, kind="ExternalInput")
    w1T_d = nc.dram_tensor("w1T", [8, 128, CB], bf16, kind="ExternalInput")
    w1b_d = nc.dram_tensor("w1b", [1, CB], bf16, kind="ExternalInput")
    woffT_d = nc.dram_tensor("woffT", [9, 2, 128, KOFF], bf16, kind="ExternalInput")
    boff_d = nc.dram_tensor("boff", [KOFF, 1], f32, kind="ExternalInput")
    w2T_d = nc.dram_tensor("w2T", [9, 2, 128, CB], bf16, kind="ExternalInput")
    b2_d = nc.dram_tensor("b2", [128, 2], f32, kind="ExternalInput")
    w3T_d = nc.dram_tensor("w3T", [2, 128, COUT], bf16, kind="ExternalInput")
    b3_d = nc.dram_tensor("b3", [128, 8], f32, kind="ExternalInput")
    basey_d = nc.dram_tensor("basey", [36, NW], f32, kind="ExternalInput")
    basex_d = nc.dram_tensor("basex", [36, NW], f32, kind="ExternalInput")
    kia_d = nc.dram_tensor("kia", [36, 1], f32, kind="ExternalInput")
    kja_d = nc.dram_tensor("kja", [36, 1], f32, kind="ExternalInput")
    vb_d = nc.dram_tensor("vb", [36, 4], f32, kind="ExternalInput")
    sel_d = nc.dram_tensor("sel", [36, 36 * 128], bf16, kind="ExternalInput")
    # idx staging: per tap padded to 48*128; xbar-transposed to wrap-128 layout
    dstg_t_d = nc.dram_tensor("dstg_t", [9, 48 * 128], i16)
    dstg_b_d = nc.dram_tensor("dstg_b", [9, 48 * 128], i16)
    rrep_t_d = nc.dram_tensor("rrep_t", [128, 432], i16)
    rrep_b_d = nc.dram_tensor("rrep_b", [128, 432], i16)
    out_d = nc.dram_tensor("out", [8, 128, NVALID], f32, kind="ExternalOutput")
    import os
    dbg = os.environ.get("KERNEL_DEBUG", "0") == "1"
    if dbg:
        dbg_idx_t = nc.dram_tensor("dbg_idx_t", [128, 8 * 9 * 48], i16,
                                   kind="ExternalOutput")
        dbg_wsb = nc.dram_tensor("dbg_wsb", [128, 4 * 640], bf16,
                                 kind="ExternalOutput")
        dbg_out2 = nc.dram_tensor("dbg_out2", [128, 2 * NPOS], bf16,
                                  kind="ExternalOutput")
        dbg_s0 = nc.dram_tensor("dbg_s0", [128, 2 * 640], bf16,
                                kind="ExternalOutput")
        dbg_ipt = nc.dram_tensor("dbg_ipt", [36, NW], i16,
                                 kind="ExternalOutput")
        dbg_om = nc.dram_tensor("dbg_om", [KOFF, NPOS], f32,
                                kind="ExternalOutput")

    with TileContext(nc) as tc:
        with tc.tile_pool(name="persist", bufs=1) as pp, \
             tc.tile_pool(name="io", bufs=3) as iop:

            tm = pp.tile([128, RSTRIP, 4, 128], bf16)
            w00 = pp.tile([36, NW], bf16)
            w01 = pp.tile([36, NW], bf16)
            w10 = pp.tile([36, NW], bf16)
            w11 = pp.tile([36, NW], bf16)
            # wrap-16 gather idx, replicated x8: [128=(8 rep,16 q), m, k, t]
            idxall_t = pp.tile([128, 8, 9, 48], i16)
            idxall_b = pp.tile([128, 8, 9, 48], i16)
            b2 = pp.tile([128, 2], f32)
            nc.sync.dma_start(out=b2, in_=b2_d[:, :])
            b3 = pp.tile([128, 8], f32)
            nc.sync.dma_start(out=b3, in_=b3_d[:, :])
            sel = pp.tile([36, 36 * 128], bf16)
            nc.sync.dma_start(out=sel, in_=sel_d[:, :])

            nc.vector.memset(tm[:, :, :, :], 0)

            with tc.tile_pool(name="omscope", bufs=1) as omp:
                om = omp.tile([KOFF, NPOS], f32)
                nc.vector.memset(om[:, :], 0)

                # ======== Stage A/B: conv1, tm build, offset conv ========
                with tc.tile_pool(name="stageab", bufs=1) as ap, \
                     tc.tile_pool(name="xck", bufs=2) as xp, \
                     tc.tile_pool(name="psA", bufs=2, space="PSUM") as psA:

                    out1_cm = ap.tile([128, 2, RSTRIP * WPAD], bf16)
                    nc.vector.memset(out1_cm[:, :, :], 0)
                    w1T = ap.tile([128, 8, CB], bf16)
                    for kt in range(8):
                        nc.sync.dma_start(out=w1T[:, kt, :], in_=w1T_d[kt, :, :])
                    w1b = ap.tile([1, CB], bf16)
                    nc.sync.dma_start(out=w1b, in_=w1b_d[:, :])
                    woffT = ap.tile([128, 9, 2, KOFF], bf16)
                    for tap in range(9):
                        for ct in range(2):
                            nc.sync.dma_start(out=woffT[:, tap, ct, :],
                                              in_=woffT_d[tap, ct, :, :])
                    boff = ap.tile([KOFF, 1], f32)
                    nc.sync.dma_start(out=boff, in_=boff_d[:, :])
                    ident = ap.tile([128, 128], bf16)
                    make_identity(nc, ident)

                    cmv = out1_cm.rearrange("p c (r w) -> p c r w", w=WPAD)

                    chunks = [(4 * i, 4) for i in range(14)] + [(56, 2)]
                    for (r0, nrows) in chunks:
                        npos = nrows * W
                        xt = xp.tile([128, 8, 4 * W], bf16, tag="xchunk")
                        for kt in range(8):
                            eng = nc.sync if kt % 2 == 0 else nc.scalar
                            eng.dma_start(out=xt[:, kt, :npos],
                                          in_=xs_d[kt, :, r0 * W:r0 * W + npos])
                        indt = xp.tile([1, 4 * W], bf16, tag="indchunk")
                        nc.sync.dma_start(out=indt[:, :npos],
                                          in_=ind_d[:, r0 * W:r0 * W + npos])
                        for mt in range(2):
                            ps = psA.tile([128, 4 * W], f32, tag="convps")
                            for kt in range(8):
                                nc.tensor.matmul(ps[:, :npos], w1T[:, kt, ts(mt, 128)],
                                                 xt[:, kt, :npos],
                                                 start=(kt == 0), stop=False)
                            nc.tensor.matmul(ps[:, :npos], w1b[:, ts(mt, 128)],
                                             indt[:, :npos], start=False, stop=True)
                            nc.scalar.activation(
                                cmv[:, mt, r0:r0 + nrows, PAD:PAD + W],
                                ps[:, :npos].rearrange("p (r w) -> p r w", w=W),
                                AF.Relu)

                    for rc in range(10):
                        r0 = rc * 5
                        npos = 5 * W
                        ps = psA.tile([KOFF, 5 * W], f32, tag="omps")
                        first = True
                        for tap in range(9):
                            ti, tj = divmod(tap, 3)
                            rhs = cmv[:, :, r0 + 3 + ti:r0 + 3 + ti + 5,
                                      PAD + tj - 1:PAD + tj - 1 + W]
                            for ct in range(2):
                                nc.tensor.matmul(
                                    ps.rearrange("p (r w) -> p r w", w=W),
                                    woffT[:, tap, ct, :], rhs[:, ct],
                                    start=first, stop=(tap == 8 and ct == 1))
                                first = False
                        nc.scalar.activation(om[:, rc * npos:(rc + 1) * npos], ps,
                                             AF.Identity, bias=boff[:, :])
                    if dbg:
                        nc.sync.dma_start(out=dbg_om[:, :], in_=om[:, :])

                    for y in range(RSTRIP):
                        for ct in range(2):
                            for sh in range(2):
                                ncols = WPAD if sh == 0 else WPAD - 1
                                pst = psA.tile([128, 128], bf16, tag="tpose")
                                nc.tensor.transpose(pst[:ncols, :],
                                                    cmv[:, ct, y, sh:sh + ncols],
                                                    ident)
                                nc.scalar.copy(tm[:ncols, y, 2 * sh + ct, :],
                                               pst[:ncols, :])

                # ======== Stage B2: packed field pipeline ========
                with tc.tile_pool(name="fieldsc", bufs=1) as fc:
                    _tc_n = [0]

                    def T(tag, d=f32):
                        _tc_n[0] += 1
                        return fc.tile([36, NW], d, tag=tag,
                                       name=f"fld_{tag}_{_tc_n[0]}")

                    dyp = T("pA")
                    dxp = T("pB")
                    mrp = T("pC")
                    basey = T("pD")
                    basex = T("pE")
                    for k in range(9):
                        nc.sync.dma_start(
                            out=dyp[4 * k:4 * k + 4, :],
                            in_=om[2 * k:2 * k + 1, :].rearrange(
                                "q (c n) -> q c n", n=NW))
                        nc.sync.dma_start(
                            out=dxp[4 * k:4 * k + 4, :],
                            in_=om[2 * k + 1:2 * k + 2, :].rearrange(
                                "q (c n) -> q c n", n=NW))
                        nc.sync.dma_start(
                            out=mrp[4 * k:4 * k + 4, :],
                            in_=om[18 + k:19 + k, :].rearrange(
                                "q (c n) -> q c n", n=NW))
                    nc.sync.dma_start(out=basey, in_=basey_d[:, :])
                    nc.sync.dma_start(out=basex, in_=basex_d[:, :])
                    kia = fc.tile([36, 1], f32)
                    nc.sync.dma_start(out=kia, in_=kia_d[:, :])
                    kja = fc.tile([36, 1], f32)
                    nc.sync.dma_start(out=kja, in_=kja_d[:, :])
                    vb = fc.tile([36, 4], f32)
                    nc.sync.dma_start(out=vb, in_=vb_d[:, :])

                    ayy = T("pF")
                    nc.vector.tensor_add(ayy, dyp, basey)          # pA,pD free
                    nc.scalar.activation(ayy, ayy, AF.Identity, bias=kia[:, :])
                    ayi = T("pA", i32)
                    nc.vector.tensor_copy(ayi, ayy)                # floor(yy)+8
                    ayf = T("pD")
                    nc.vector.tensor_copy(ayf, ayi)                # pA free
                    wyh = T("pG")                                  # wy - 0.5
                    nc.vector.tensor_sub(wyh, ayy, ayf)            # pF free
                    msig = T("pF")
                    nc.scalar.activation(msig, mrp, AF.Sigmoid)    # pC free
                    bxx = T("pC")
                    nc.vector.tensor_add(bxx, dxp, basex)          # pB,pE free
                    nc.scalar.activation(bxx, bxx, AF.Identity, bias=kja[:, :])
                    bxi = T("pB", i32)
                    nc.vector.tensor_copy(bxi, bxx)
                    bxf = T("pE")
                    nc.vector.tensor_copy(bxf, bxi)                # pB free
                    wxh = T("pH")
                    nc.vector.tensor_sub(wxh, bxx, bxf)            # pC free

                    # gather indices from floors (ayf, bxf live)
                    idxpf = T("pB")
                    nc.vector.tensor_scalar(idxpf, ayf, 128.0, -1032.0,
                                            ALU.mult, ALU.add)
                    nc.vector.tensor_add(idxpf, idxpf, bxf)
                    idx_pt = fc.tile([36, NW], i16, tag="pI1")
                    nc.vector.tensor_copy(idx_pt, idxpf)
                    nc.vector.tensor_scalar(idxpf, idxpf, 128.0, None, ALU.add)
                    idx_pb = fc.tile([36, NW], i16, tag="pI2")
                    nc.vector.tensor_copy(idx_pb, idxpf)           # pB free
                    # relayout packed p-order -> wrap-128 via xbar DMA
                    # transpose, then replicate x8 via DRAM round trip.
                    if dbg:
                        nc.sync.dma_start(out=dbg_ipt[:, :], in_=idx_pt[:, :])
                    zpad = fc.tile([9, 1024], i16, tag="zpad")
                    nc.vector.memset(zpad, 0)
                    for wi, (idx_p, dstg_d, rrep_d, idxall) in enumerate((
                            (idx_pt, dstg_t_d, rrep_t_d, idxall_t),
                            (idx_pb, dstg_b_d, rrep_b_d, idxall_b))):
                        nc.sync.dma_start(out=dstg_d[:, NPOS:], in_=zpad[:, :])
                        for k in range(9):
                            nc.sync.dma_start(
                                out=dstg_d[k, :NPOS].rearrange(
                                    "(c n) -> c n", n=NW),
                                in_=idx_p[4 * k:4 * k + 4, :])
                        w128 = fc.tile([128, 432], i16, tag="w128",
                                       name=f"w128_{wi}")
                        nc.sync.dma_start(
                            out=w128[:, :],
                            in_=dstg_d.rearrange("k (r x) -> (k r) x", x=128),
                            transpose=True)
                        nc.sync.dma_start(out=rrep_d[:, :], in_=w128[:, :])
                        for g in range(8):
                            nc.sync.dma_start(
                                out=idxall[16 * g:16 * g + 16, :, :, :],
                                in_=rrep_d.rearrange(
                                    "(m q) (k t) -> q m k t", q=16, t=48))

                    # validity + weight maps
                    def cmp_range(dst, src, lo_ap, hi_ap, tmp):
                        nc.vector.tensor_scalar(tmp, src, lo_ap, None, ALU.is_ge)
                        nc.vector.tensor_scalar(dst, src, hi_ap, None, ALU.is_le)
                        nc.vector.tensor_mul(dst, dst, tmp)

                    tmp = T("pA")
                    vy0 = T("pB")
                    cmp_range(vy0, ayf, vb[:, 0:1], vb[:, 1:2], tmp)
                    vy1 = T("pC")
                    cmp_range(vy1, ayf, vb[:, 2:3], vb[:, 3:4], tmp)   # pD free
                    atop = T("pD")
                    nc.vector.tensor_scalar(atop, wyh, -1.0, 0.5, ALU.mult, ALU.add)
                    nc.vector.tensor_mul(atop, atop, msig)
                    nc.vector.tensor_mul(atop, atop, vy0)              # pB free
                    abot = T("pB")
                    nc.vector.tensor_scalar(abot, wyh, 0.5, None, ALU.add)
                    nc.vector.tensor_mul(abot, abot, msig)
                    nc.vector.tensor_mul(abot, abot, vy1)   # pG, pF, pC free
                    vx0 = T("pC")
                    cmp_range(vx0, bxf, 12.0, 111.0, tmp)
                    vx1 = T("pF")
                    cmp_range(vx1, bxf, 11.0, 110.0, tmp)              # pE free
                    c0 = T("pE")
                    nc.vector.tensor_scalar(c0, wxh, -1.0, 0.5, ALU.mult, ALU.add)
                    nc.vector.tensor_mul(c0, c0, vx0)                  # pC free
                    c1 = T("pC")
                    nc.vector.tensor_scalar(c1, wxh, 0.5, None, ALU.add)
                    nc.vector.tensor_mul(c1, c1, vx1)                  # pH, pF free
                    nc.vector.tensor_mul(w00, atop, c0)
                    nc.vector.tensor_mul(w01, atop, c1)
                    nc.vector.tensor_mul(w10, abot, c0)
                    nc.vector.tensor_mul(w11, abot, c1)

            # ======== Stage C + D ========
            # chunk m in 0..7 covers interleaved positions p = 128*s + 16*m + q
            # (s in 0..39, q in 0..15); gather slot i = 16*s + q.
            MCH = 640
            with tc.tile_pool(name="stagecd", bufs=1) as cp, \
                 tc.tile_pool(name="gath", bufs=3) as gp, \
                 tc.tile_pool(name="psC", bufs=1, space="PSUM") as psC, \
                 tc.tile_pool(name="psW", bufs=2, space="PSUM") as psW:

                out2 = cp.tile([128, 2, NPOS], bf16)
                o2v = out2.rearrange("p c (s x) -> p c s x", x=128)
                w2T = cp.tile([128, 9, 2, CB], bf16)
                for k in range(9):
                    for ct in range(2):
                        nc.sync.dma_start(out=w2T[:, k, ct, :], in_=w2T_d[k, ct, :, :])

                tmflat = tm.rearrange("p r a b -> p (r a b)")
                wmaps = (w00, w01, w10, w11)
                wvs = [w.rearrange("p (s x) -> p s x", x=128) for w in wmaps]
                gq = [0]  # SWDGE queue round-robin, aligned with sem lanes
                for m in range(8):
                    accA = [psC.tile([128, 512], f32, tag=f"daccA{mt}",
                                     name=f"daccA_{m}_{mt}") for mt in range(2)]
                    accB = [psC.tile([128, 128], f32, tag=f"daccB{mt}",
                                     name=f"daccB_{m}_{mt}") for mt in range(2)]
                    for k in range(9):
                        gt = gp.tile([128, 4, MCH], bf16, tag="gtop", bufs=4)
                        gb = gp.tile([128, 4, MCH], bf16, tag="gbot", bufs=4)
                        for (gx, idxall) in ((gt, idxall_t), (gb, idxall_b)):
                            nc.gpsimd.dma_gather(
                                out_ap=gx[:, :, :], in_ap=tmflat,
                                idxs_ap=idxall[:, m, k, 0:40],
                                num_idxs=MCH, num_idxs_reg=MCH,
                                elem_size=512, transpose=True,
                                sbuf_tokens_per_rank=128,
                                sbuf_free_dim_per_rank=1024,
                                queue_num=gq[0] % 4)
                            gq[0] += 1
                        wrep = []
                        for ci, wv in enumerate(wvs):
                            rhs = wv[:, :, 16 * m:16 * m + 16]
                            pwA = psW.tile([128, 512], f32, tag="wpsA",
                                           name=f"wpsA_{m}_{k}_{ci}")
                            pwB = psW.tile([128, 128], f32, tag="wpsB",
                                           name=f"wpsB_{m}_{k}_{ci}")
                            pAv = pwA.rearrange("p (s x) -> p s x", x=16)
                            for c in range(3):
                                nc.tensor.matmul(
                                    pAv[:, 10 * c:10 * c + 10, :],
                                    sel[:, (4 * k + c) * 128:(4 * k + c + 1) * 128],
                                    rhs, start=True, stop=True)
                            sel3 = sel[:, (4 * k + 3) * 128:(4 * k + 4) * 128]
                            nc.tensor.matmul(pAv[:, 30:32, :], sel3,
                                             rhs[:, 0:2, :], start=True, stop=True)
                            nc.tensor.matmul(
                                pwB.rearrange("p (s x) -> p s x", x=16),
                                sel3, rhs[:, 2:10, :], start=True, stop=True)
                            wsb = gp.tile([128, MCH], bf16, tag=f"wsb{ci}",
                                          name=f"wsb_{m}_{k}_{ci}", bufs=2)
                            nc.scalar.copy(wsb[:, 0:512], pwA)
                            nc.scalar.copy(wsb[:, 512:640], pwB)
                            wrep.append(wsb)
                        s0 = gp.tile([128, 2, MCH], bf16, tag="s0")
                        s1 = gp.tile([128, 2, MCH], bf16, tag="s1")

                        def bc(wsb):
                            return wsb.unsqueeze(1).broadcast_to([128, 2, MCH])

                        nc.vector.tensor_mul(s0, gt[:, 0:2, :], bc(wrep[0]))
                        nc.vector.tensor_mul(s1, gt[:, 2:4, :], bc(wrep[1]))
                        nc.vector.tensor_add(s0, s0, s1)
                        nc.vector.tensor_mul(s1, gb[:, 0:2, :], bc(wrep[2]))
                        nc.vector.tensor_add(s0, s0, s1)
                        nc.vector.tensor_mul(s1, gb[:, 2:4, :], bc(wrep[3]))
                        nc.vector.tensor_add(s0, s0, s1)
                        if dbg and m == 3 and k == 0:
                            for ci in range(4):
                                nc.sync.dma_start(
                                    out=dbg_wsb[:, 640 * ci:640 * (ci + 1)],
                                    in_=wrep[ci][:, :])
                            nc.sync.dma_start(
                                out=dbg_s0[:, :].rearrange(
                                    "p (c n) -> p c n", n=MCH),
                                in_=s0[:, :, :])
                        for mt in range(2):
                            for ct in range(2):
                                st = (k == 0 and ct == 0)
                                sp = (k == 8 and ct == 1)
                                w2s = w2T[:, k, ct, ts(mt, 128)]
                                nc.tensor.matmul(accA[mt], w2s, s0[:, ct, 0:512],
                                                 start=st, stop=sp)
                                nc.tensor.matmul(accB[mt], w2s, s0[:, ct, 512:640],
                                                 start=st, stop=sp)
                    for mt in range(2):
                        nc.scalar.activation(
                            o2v[:, mt, 0:32, 16 * m:16 * m + 16],
                            accA[mt].rearrange("p (s x) -> p s x", x=16),
                            AF.Relu, bias=b2[:, mt:mt + 1])
                        nc.scalar.activation(
                            o2v[:, mt, 32:40, 16 * m:16 * m + 16],
                            accB[mt].rearrange("p (s x) -> p s x", x=16),
                            AF.Relu, bias=b2[:, mt:mt + 1])

                if dbg:
                    nc.sync.dma_start(
                        out=dbg_idx_t[:, :].rearrange(
                            "p (m k t) -> p m k t", k=9, t=48),
                        in_=idxall_t[:, :, :, :])
                    nc.sync.dma_start(
                        out=dbg_out2[:, :].rearrange("p (c n) -> p c n", n=NPOS),
                        in_=out2[:, :, :])

                # ---- Stage D ----
                w3T = cp.tile([128, 2, COUT], bf16)
                for ct in range(2):
                    nc.sync.dma_start(out=w3T[:, ct, :], in_=w3T_d[ct, :, :])
                for ch in range(NCHUNKS):
                    n0 = ch * NCHUNK
                    nn = min(NCHUNK, NVALID - n0)
                    if nn <= 0:
                        break
                    for mt in range(8):
                        xr = iop.tile([128, NCHUNK], bf16, tag="xres")
                        nc.sync.dma_start(out=xr[:, :nn],
                                          in_=xs_d[mt, :, PAD * W + n0:PAD * W + n0 + nn])
                        ps = psC.tile([128, NCHUNK], f32, tag=f"daccA{mt % 2}",
                                      name=f"c3ps_{ch}_{mt}")
                        for ct in range(2):
                            nc.tensor.matmul(ps[:, :nn], w3T[:, ct, ts(mt, 128)],
                                             out2[:, ct, n0:n0 + nn],
                                             start=(ct == 0), stop=(ct == 1))
                        t = iop.tile([128, NCHUNK], f32, tag="resid")
                        nc.vector.tensor_add(t[:, :nn], ps[:, :nn], xr[:, :nn])
                        o = iop.tile([128, NCHUNK], f32, tag="obuf")
                        nc.scalar.activation(o[:, :nn], t[:, :nn], AF.Relu,
                                             bias=b3[:, mt:mt + 1])
                        nc.scalar.dma_start(out=out_d[mt, :, n0:n0 + nn],
                                            in_=o[:, :nn])

    nc.finalize()
    return nc


_NC_CACHE = None


def _get_nc():
    global _NC_CACHE
    if _NC_CACHE is None:
        _NC_CACHE = _build_program()
    return _NC_CACHE


def _prep_inputs(x, w1, s1, b1, w_off, b_off, w2, s2, b2, w3, s3, b3):
    bf16 = ml_dtypes.bfloat16
    f32 = np.float32
    x = np.asarray(x, f32)
    w1f = np.asarray(w1, f32) * np.asarray(s1, f32)[:, None]
    w1T = np.ascontiguousarray(w1f.T.reshape(8, 128, CB)).astype(bf16)
    w1b = np.ascontiguousarray(np.asarray(b1, f32)[None, :]).astype(bf16)
    woffT = np.zeros((9, 2, 128, KOFF), bf16)
    w_off = np.asarray(w_off, f32)
    for tap in range(9):
        ti, tj = divmod(tap, 3)
        wt = w_off[:, :, ti, tj]
        for ct in range(2):
            woffT[tap, ct] = wt[:, ct * 128:(ct + 1) * 128].T.astype(bf16)
    boff = np.asarray(b_off, f32)[:, None]
    w2f = np.asarray(w2, f32) * np.asarray(s2, f32)[:, None, None, None]
    w2T = np.zeros((9, 2, 128, CB), bf16)
    for k in range(9):
        ki, kj = divmod(k, 3)
        wk = w2f[:, :, ki, kj]
        for ct in range(2):
            w2T[k, ct] = wk[:, ct * 128:(ct + 1) * 128].T.astype(bf16)
    b2t = np.ascontiguousarray(np.asarray(b2, f32).reshape(2, 128).T)
    w3f = np.asarray(w3, f32) * np.asarray(s3, f32)[:, None]
    w3T = np.zeros((2, 128, COUT), bf16)
    for ct in range(2):
        w3T[ct] = w3f[:, ct * 128:(ct + 1) * 128].T.astype(bf16)
    b3t = np.ascontiguousarray(np.asarray(b3, f32).reshape(8, 128).T)

    j = np.arange(NPOS)
    y_loc = np.where(j < NVALID, PAD + j // W, 20).astype(f32)
    x_pad = np.where(j < NVALID, PAD + j % W, 50).astype(f32)
    basey = np.zeros((36, NW), f32)
    basex = np.zeros((36, NW), f32)
    kia = np.zeros((36, 1), f32)
    kja = np.zeros((36, 1), f32)
    for k in range(9):
        ki, kj = divmod(k, 3)
        for c in range(4):
            basey[4 * k + c] = y_loc[c * NW:(c + 1) * NW]
            basex[4 * k + c] = x_pad[c * NW:(c + 1) * NW]
            kia[4 * k + c] = ki - 1 + 7.5
            kja[4 * k + c] = kj - 1 + 7.5
    selm = np.zeros((36, 36 * 128), bf16)
    for r in range(36):
        selm[r, r * 128:(r + 1) * 128] = 1.0

    shared = dict(w1T=w1T, w1b=w1b, woffT=np.asarray(woffT), boff=boff,
                  w2T=np.asarray(w2T), b2=b2t, w3T=np.asarray(w3T), b3=b3t,
                  basey=basey, basex=basex, kia=kia, kja=kja,
                  sel=np.asarray(selm))

    in_maps = []
    for core in range(8):
        b, half = core // 2, core % 2
        lo = half * 50
        xs = np.zeros((CIN, RSTRIP, W), f32)
        vlo = max(0, lo - PAD)
        vhi = min(H - 1, lo + 49 + PAD)
        loc0 = vlo - (lo - PAD)
        nrows = vhi - vlo + 1
        xs[:, loc0:loc0 + nrows, :] = x[b, :, vlo:vhi + 1, :]
        indv = np.zeros((RSTRIP, W), f32)
        indv[loc0:loc0 + nrows, :] = 1.0
        vbm = np.zeros((36, 4), f32)
        vbm[:, 0] = loc0 + 8
        vbm[:, 1] = loc0 + nrows - 1 + 8
        vbm[:, 2] = loc0 + 8 - 1
        vbm[:, 3] = loc0 + nrows - 1 + 8 - 1
        in_maps.append(dict(shared,
                            xs=np.ascontiguousarray(
                                xs.reshape(8, 128, RSTRIP * W)).astype(bf16),
                            ind=indv.reshape(1, -1).astype(bf16), vb=vbm))
    return in_maps


def kernel(**inputs):
    from concourse.bass_utils import run_bass_kernel_spmd
    nc = _get_nc()
    in_maps = _prep_inputs(**inputs)
    run_bass_kernel_spmd(nc, in_maps, core_ids=list(range(8)))  # warm-up
    res = run_bass_kernel_spmd(nc, in_maps, core_ids=list(range(8)))
    out = np.zeros((B, COUT, H, W), np.float32)
    for core in range(8):
        b, half = core // 2, core % 2
        lo = half * 50
        o = res.results[core]["out"].reshape(COUT, 50, W)
        out[b, :, lo:lo + 50, :] = o
    return out



# revision 40
# speedup vs baseline: 3.1731x; 1.0140x over previous
"""Trainium2 Bass kernel for DeformBottleneckBlock (DCNv2 bottleneck).

Sharding: 8 cores = (batch b in 0..3) x (H-half in 0..1); each core computes
output rows [lo, lo+50) of one image. Fully data-parallel, no collectives.

Per-core pipeline:
  A) conv1 1x1 (fp32, bn1 folded, bias via indicator channel so out-of-image
     rows are exactly zero) -> out1 channel-major bf16, padded strip
     [128, 2, 58*108]; then PE transposes build a shingled token-major buffer
     tm[x_pad, y, 512ch] (own 256 ch + right neighbor 256 ch, 1KB/token).
  B) offset conv 3x3 (im2col shifted views, PSUM-accumulated) -> om [27,5120].
     Field pipeline on packed [36,1280] tiles (partition = 4k+chunk):
     bilinear corner weight maps W00..11 (validity-masked, mask-folded) and
     int16 gather indices (floor via round(x+7.5) cast).
     Idx relayout to the gather's wrap-16 format without tiny-descriptor
     DMAs: contiguous DRAM staging [9,48*128] -> one xbar DMA-transpose
     -> wrap-128 [128,432] -> DRAM round trip replicates x8 -> idxall.
  C) chunks are wrap-interleaved: chunk m (640 slots) covers positions
     p = 128*s + 16*m + q so idx tiles are contiguous idxall slices.
     Per (k,m): two dma_gathers (corner row pairs, 4 SWDGE queues,
     sem-lane-aligned round-robin), sel-matmul broadcast of the 4 weight
     maps via strided rhs views (4 row-blocks + tail), 4 muls + 3 adds ->
     s_k bf16, PSUM-accumulated w2 matmuls (512+128 col splits), bn2 bias
     + relu written back in p-order via strided activation dst views.
  D) conv3 1x1 (bf16) + residual add (bf16 x) + bn3 bias + relu -> out.
"""

import numpy as np
import ml_dtypes

B, CIN, H, W = 4, 1024, 100, 100
CB, COUT, KOFF = 256, 1024, 27

PAD = 4
RSTRIP = 58
WPAD = 108
NPOS = 5120
NCHUNK = 512
NCHUNKS = NPOS // NCHUNK
NW = 1280
NVALID = 5000


def _build_program():
    import concourse.bacc as bacc
    import concourse.mybir as mybir
    from concourse.tile import TileContext
    from concourse.bass import ts
    from concourse.masks import make_identity

    dt = mybir.dt
    AF = mybir.ActivationFunctionType
    ALU = mybir.AluOpType
    f32, bf16, i16, i32 = dt.float32, dt.bfloat16, dt.int16, dt.int32

    nc = bacc.Bacc("TRN2", target_bir_lowering=False, num_swdge_queues=4)

    xs_d = nc.dram_tensor("xs", [8, 128, RSTRIP * W], bf16, kind="ExternalInput")
    ind_d = nc.dram_tensor("ind", [1, RSTRIP * W], bf16, kind="ExternalInput")
    w1T_d = nc.dram_tensor("w1T", [8, 128, CB], bf16, kind="ExternalInput")
    w1b_d = nc.dram_tensor("w1b", [1, CB], bf16, kind="ExternalInput")
    woffT_d = nc.dram_tensor("woffT", [9, 2, 128, KOFF], bf16, kind="ExternalInput")
    boff_d = nc.dram_tensor("boff", [KOFF, 1], f32, kind="ExternalInput")
    w2T_d = nc.dram_tensor("w2T", [9, 2, 128, CB], bf16, kind="ExternalInput")
    b2_d = nc.dram_tensor("b2", [128, 2], f32, kind="ExternalInput")
    w3T_d = nc.dram_tensor("w3T", [2, 128, COUT], bf16, kind="ExternalInput")
    b3_d = nc.dram_tensor("b3", [128, 8], f32, kind="ExternalInput")
    basey_d = nc.dram_tensor("basey", [36, NW], f32, kind="ExternalInput")
    basex_d = nc.dram_tensor("basex", [36, NW], f32, kind="ExternalInput")
    kia_d = nc.dram_tensor("kia", [36, 1], f32, kind="ExternalInput")
    kja_d = nc.dram_tensor("kja", [36, 1], f32, kind="ExternalInput")
    vb_d = nc.dram_tensor("vb", [36, 4], f32, kind="ExternalInput")
    sel_d = nc.dram_tensor("sel", [36, 36 * 128], bf16, kind="ExternalInput")
    # idx staging: per tap padded to 48*128; xbar-transposed to wrap-128 layout
    dstg_t_d = nc.dram_tensor("dstg_t", [9, 48 * 128], i16)
    dstg_b_d = nc.dram_tensor("dstg_b", [9, 48 * 128], i16)
    rrep_t_d = nc.dram_tensor("rrep_t", [128, 432], i16)
    rrep_b_d = nc.dram_tensor("rrep_b", [128, 432], i16)
    out_d = nc.dram_tensor("out", [8, 128, NVALID], bf16, kind="ExternalOutput")
    import os
    dbg = os.environ.get("KERNEL_DEBUG", "0") == "1"
    if dbg:
        dbg_idx_t = nc.dram_tensor("dbg_idx_t", [128, 8 * 9 * 48], i16,
                                   kind="ExternalOutput")
        dbg_wsb = nc.dram_tensor("dbg_wsb", [128, 4 * 640], bf16,
                                 kind="ExternalOutput")
        dbg_out2 = nc.dram_tensor("dbg_out2", [128, 2 * NPOS], bf16,
                                  kind="ExternalOutput")
        dbg_s0 = nc.dram_tensor("dbg_s0", [128, 2 * 640], bf16,
                                kind="ExternalOutput")
        dbg_ipt = nc.dram_tensor("dbg_ipt", [36, NW], i16,
                                 kind="ExternalOutput")
        dbg_om = nc.dram_tensor("dbg_om", [KOFF, NPOS], f32,
                                kind="ExternalOutput")

    with TileContext(nc) as tc:
        with tc.tile_pool(name="persist", bufs=1) as pp, \
             tc.tile_pool(name="io", bufs=3) as iop:

            tm = pp.tile([128, RSTRIP, 4, 128], bf16)
            w00 = pp.tile([36, NW], bf16)
            w01 = pp.tile([36, NW], bf16)
            w10 = pp.tile([36, NW], bf16)
            w11 = pp.tile([36, NW], bf16)
            # wrap-16 gather idx, replicated x8: [128=(8 rep,16 q), m, k, t]
            idxall_t = pp.tile([128, 8, 9, 48], i16)
            idxall_b = pp.tile([128, 8, 9, 48], i16)
            b2 = pp.tile([128, 2], f32)
            nc.sync.dma_start(out=b2, in_=b2_d[:, :])
            b3 = pp.tile([128, 8], f32)
            nc.sync.dma_start(out=b3, in_=b3_d[:, :])
            sel = pp.tile([36, 36 * 128], bf16)
            nc.sync.dma_start(out=sel, in_=sel_d[:, :])

            nc.vector.memset(tm[:, :, :, :], 0)

            with tc.tile_pool(name="omscope", bufs=1) as omp:
                om = omp.tile([KOFF, NPOS], f32)
                nc.vector.memset(om[:, :], 0)

                # ======== Stage A/B: conv1, tm build, offset conv ========
                with tc.tile_pool(name="stageab", bufs=1) as ap, \
                     tc.tile_pool(name="xck", bufs=2) as xp, \
                     tc.tile_pool(name="psA", bufs=2, space="PSUM") as psA:

                    out1_cm = ap.tile([128, 2, RSTRIP * WPAD], bf16)
                    nc.vector.memset(out1_cm[:, :, :], 0)
                    w1T = ap.tile([128, 8, CB], bf16)
                    for kt in range(8):
                        nc.sync.dma_start(out=w1T[:, kt, :], in_=w1T_d[kt, :, :])
                    w1b = ap.tile([1, CB], bf16)
                    nc.sync.dma_start(out=w1b, in_=w1b_d[:, :])
                    woffT = ap.tile([128, 9, 2, KOFF], bf16)
                    for tap in range(9):
                        for ct in range(2):
                            nc.sync.dma_start(out=woffT[:, tap, ct, :],
                                              in_=woffT_d[tap, ct, :, :])
                    boff = ap.tile([KOFF, 1], f32)
                    nc.sync.dma_start(out=boff, in_=boff_d[:, :])
                    ident = ap.tile([128, 128], bf16)
                    make_identity(nc, ident)

                    cmv = out1_cm.rearrange("p c (r w) -> p c r w", w=WPAD)

                    chunks = [(4 * i, 4) for i in range(14)] + [(56, 2)]
                    for (r0, nrows) in chunks:
                        npos = nrows * W
                        xt = xp.tile([128, 8, 4 * W], bf16, tag="xchunk")
                        for kt in range(8):
                            eng = nc.sync if kt % 2 == 0 else nc.scalar
                            eng.dma_start(out=xt[:, kt, :npos],
                                          in_=xs_d[kt, :, r0 * W:r0 * W + npos])
                        indt = xp.tile([1, 4 * W], bf16, tag="indchunk")
                        nc.sync.dma_start(out=indt[:, :npos],
                                          in_=ind_d[:, r0 * W:r0 * W + npos])
                        for mt in range(2):
                            ps = psA.tile([128, 4 * W], f32, tag="convps")
                            for kt in range(8):
                                nc.tensor.matmul(ps[:, :npos], w1T[:, kt, ts(mt, 128)],
                                                 xt[:, kt, :npos],
                                                 start=(kt == 0), stop=False)
                            nc.tensor.matmul(ps[:, :npos], w1b[:, ts(mt, 128)],
                                             indt[:, :npos], start=False, stop=True)
                            nc.scalar.activation(
                                cmv[:, mt, r0:r0 + nrows, PAD:PAD + W],
                                ps[:, :npos].rearrange("p (r w) -> p r w", w=W),
                                AF.Relu)

                    for rc in range(10):
                        r0 = rc * 5
                        npos = 5 * W
                        ps = psA.tile([KOFF, 5 * W], f32, tag="omps")
                        first = True
                        for tap in range(9):
                            ti, tj = divmod(tap, 3)
                            rhs = cmv[:, :, r0 + 3 + ti:r0 + 3 + ti + 5,
                                      PAD + tj - 1:PAD + tj - 1 + W]
                            for ct in range(2):
                                nc.tensor.matmul(
                                    ps.rearrange("p (r w) -> p r w", w=W),
                                    woffT[:, tap, ct, :], rhs[:, ct],
                                    start=first, stop=(tap == 8 and ct == 1))
                                first = False
                        nc.scalar.activation(om[:, rc * npos:(rc + 1) * npos], ps,
                                             AF.Identity, bias=boff[:, :])
                    if dbg:
                        nc.sync.dma_start(out=dbg_om[:, :], in_=om[:, :])

                    for y in range(RSTRIP):
                        for ct in range(2):
                            for sh in range(2):
                                ncols = WPAD if sh == 0 else WPAD - 1
                                pst = psA.tile([128, 128], bf16, tag="tpose")
                                nc.tensor.transpose(pst[:ncols, :],
                                                    cmv[:, ct, y, sh:sh + ncols],
                                                    ident)
                                nc.scalar.copy(tm[:ncols, y, 2 * sh + ct, :],
                                               pst[:ncols, :])

                # ======== Stage B2: packed field pipeline ========
                with tc.tile_pool(name="fieldsc", bufs=1) as fc:
                    _tc_n = [0]

                    def T(tag, d=f32):
                        _tc_n[0] += 1
                        return fc.tile([36, NW], d, tag=tag,
                                       name=f"fld_{tag}_{_tc_n[0]}")

                    dyp = T("pA")
                    dxp = T("pB")
                    mrp = T("pC")
                    basey = T("pD")
                    basex = T("pE")
                    for k in range(9):
                        nc.sync.dma_start(
                            out=dyp[4 * k:4 * k + 4, :],
                            in_=om[2 * k:2 * k + 1, :].rearrange(
                                "q (c n) -> q c n", n=NW))
                        nc.sync.dma_start(
                            out=dxp[4 * k:4 * k + 4, :],
                            in_=om[2 * k + 1:2 * k + 2, :].rearrange(
                                "q (c n) -> q c n", n=NW))
                        nc.sync.dma_start(
                            out=mrp[4 * k:4 * k + 4, :],
                            in_=om[18 + k:19 + k, :].rearrange(
                                "q (c n) -> q c n", n=NW))
                    nc.sync.dma_start(out=basey, in_=basey_d[:, :])
                    nc.sync.dma_start(out=basex, in_=basex_d[:, :])
                    kia = fc.tile([36, 1], f32)
                    nc.sync.dma_start(out=kia, in_=kia_d[:, :])
                    kja = fc.tile([36, 1], f32)
                    nc.sync.dma_start(out=kja, in_=kja_d[:, :])
                    vb = fc.tile([36, 4], f32)
                    nc.sync.dma_start(out=vb, in_=vb_d[:, :])

                    ayy = T("pF")
                    nc.vector.tensor_add(ayy, dyp, basey)          # pA,pD free
                    nc.scalar.activation(ayy, ayy, AF.Identity, bias=kia[:, :])
                    ayi = T("pA", i32)
                    nc.vector.tensor_copy(ayi, ayy)                # floor(yy)+8
                    ayf = T("pD")
                    nc.vector.tensor_copy(ayf, ayi)                # pA free
                    wyh = T("pG")                                  # wy - 0.5
                    nc.vector.tensor_sub(wyh, ayy, ayf)            # pF free
                    msig = T("pF")
                    nc.scalar.activation(msig, mrp, AF.Sigmoid)    # pC free
                    bxx = T("pC")
                    nc.vector.tensor_add(bxx, dxp, basex)          # pB,pE free
                    nc.scalar.activation(bxx, bxx, AF.Identity, bias=kja[:, :])
                    bxi = T("pB", i32)
                    nc.vector.tensor_copy(bxi, bxx)
                    bxf = T("pE")
                    nc.vector.tensor_copy(bxf, bxi)                # pB free
                    wxh = T("pH")
                    nc.vector.tensor_sub(wxh, bxx, bxf)            # pC free

                    # gather indices from floors (ayf, bxf live)
                    idxpf = T("pB")
                    nc.vector.tensor_scalar(idxpf, ayf, 128.0, -1032.0,
                                            ALU.mult, ALU.add)
                    nc.vector.tensor_add(idxpf, idxpf, bxf)
                    idx_pt = fc.tile([36, NW], i16, tag="pI1")
                    nc.vector.tensor_copy(idx_pt, idxpf)
                    nc.vector.tensor_scalar(idxpf, idxpf, 128.0, None, ALU.add)
                    idx_pb = fc.tile([36, NW], i16, tag="pI2")
                    nc.vector.tensor_copy(idx_pb, idxpf)           # pB free
                    # relayout packed p-order -> wrap-128 via xbar DMA
                    # transpose, then replicate x8 via DRAM round trip.
                    if dbg:
                        nc.sync.dma_start(out=dbg_ipt[:, :], in_=idx_pt[:, :])
                    zpad = fc.tile([9, 1024], i16, tag="zpad")
                    nc.vector.memset(zpad, 0)
                    for wi, (idx_p, dstg_d, rrep_d, idxall) in enumerate((
                            (idx_pt, dstg_t_d, rrep_t_d, idxall_t),
                            (idx_pb, dstg_b_d, rrep_b_d, idxall_b))):
                        nc.sync.dma_start(out=dstg_d[:, NPOS:], in_=zpad[:, :])
                        for k in range(9):
                            nc.sync.dma_start(
                                out=dstg_d[k, :NPOS].rearrange(
                                    "(c n) -> c n", n=NW),
                                in_=idx_p[4 * k:4 * k + 4, :])
                        w128 = fc.tile([128, 432], i16, tag="w128",
                                       name=f"w128_{wi}")
                        nc.sync.dma_start(
                            out=w128[:, :],
                            in_=dstg_d.rearrange("k (r x) -> (k r) x", x=128),
                            transpose=True)
                        nc.sync.dma_start(out=rrep_d[:, :], in_=w128[:, :])
                        for g in range(8):
                            nc.sync.dma_start(
                                out=idxall[16 * g:16 * g + 16, :, :, :],
                                in_=rrep_d.rearrange(
                                    "(m q) (k t) -> q m k t", q=16, t=48))

                    # validity + weight maps
                    def cmp_range(dst, src, lo_ap, hi_ap, tmp):
                        nc.vector.tensor_scalar(tmp, src, lo_ap, None, ALU.is_ge)
                        nc.vector.tensor_scalar(dst, src, hi_ap, None, ALU.is_le)
                        nc.vector.tensor_mul(dst, dst, tmp)

                    tmp = T("pA")
                    vy0 = T("pB")
                    cmp_range(vy0, ayf, vb[:, 0:1], vb[:, 1:2], tmp)
                    vy1 = T("pC")
                    cmp_range(vy1, ayf, vb[:, 2:3], vb[:, 3:4], tmp)   # pD free
                    atop = T("pD")
                    nc.vector.tensor_scalar(atop, wyh, -1.0, 0.5, ALU.mult, ALU.add)
                    nc.vector.tensor_mul(atop, atop, msig)
                    nc.vector.tensor_mul(atop, atop, vy0)              # pB free
                    abot = T("pB")
                    nc.vector.tensor_scalar(abot, wyh, 0.5, None, ALU.add)
                    nc.vector.tensor_mul(abot, abot, msig)
                    nc.vector.tensor_mul(abot, abot, vy1)   # pG, pF, pC free
                    vx0 = T("pC")
                    cmp_range(vx0, bxf, 12.0, 111.0, tmp)
                    vx1 = T("pF")
                    cmp_range(vx1, bxf, 11.0, 110.0, tmp)              # pE free
                    c0 = T("pE")
                    nc.vector.tensor_scalar(c0, wxh, -1.0, 0.5, ALU.mult, ALU.add)
                    nc.vector.tensor_mul(c0, c0, vx0)                  # pC free
                    c1 = T("pC")
                    nc.vector.tensor_scalar(c1, wxh, 0.5, None, ALU.add)
                    nc.vector.tensor_mul(c1, c1, vx1)                  # pH, pF free
                    nc.vector.tensor_mul(w00, atop, c0)
                    nc.vector.tensor_mul(w01, atop, c1)
                    nc.vector.tensor_mul(w10, abot, c0)
                    nc.vector.tensor_mul(w11, abot, c1)

            # ======== Stage C + D ========
            # chunk m in 0..7 covers interleaved positions p = 128*s + 16*m + q
            # (s in 0..39, q in 0..15); gather slot i = 16*s + q.
            MCH = 640
            with tc.tile_pool(name="stagecd", bufs=1) as cp, \
                 tc.tile_pool(name="gath", bufs=3) as gp, \
                 tc.tile_pool(name="psC", bufs=1, space="PSUM") as psC, \
                 tc.tile_pool(name="psW", bufs=2, space="PSUM") as psW:

                out2 = cp.tile([128, 2, NPOS], bf16)
                o2v = out2.rearrange("p c (s x) -> p c s x", x=128)
                w2T = cp.tile([128, 9, 2, CB], bf16)
                for k in range(9):
                    for ct in range(2):
                        nc.sync.dma_start(out=w2T[:, k, ct, :], in_=w2T_d[k, ct, :, :])

                tmflat = tm.rearrange("p r a b -> p (r a b)")
                wmaps = (w00, w01, w10, w11)
                wvs = [w.rearrange("p (s x) -> p s x", x=128) for w in wmaps]
                gq = [0]  # SWDGE queue round-robin, aligned with sem lanes
                for m in range(8):
                    accA = [psC.tile([128, 512], f32, tag=f"daccA{mt}",
                                     name=f"daccA_{m}_{mt}") for mt in range(2)]
                    accB = [psC.tile([128, 128], f32, tag=f"daccB{mt}",
                                     name=f"daccB_{m}_{mt}") for mt in range(2)]
                    for k in range(9):
                        gt = gp.tile([128, 4, MCH], bf16, tag="gtop", bufs=4)
                        gb = gp.tile([128, 4, MCH], bf16, tag="gbot", bufs=4)
                        for (gx, idxall) in ((gt, idxall_t), (gb, idxall_b)):
                            nc.gpsimd.dma_gather(
                                out_ap=gx[:, :, :], in_ap=tmflat,
                                idxs_ap=idxall[:, m, k, 0:40],
                                num_idxs=MCH, num_idxs_reg=MCH,
                                elem_size=512, transpose=True,
                                sbuf_tokens_per_rank=128,
                                sbuf_free_dim_per_rank=1024,
                                queue_num=gq[0] % 4)
                            gq[0] += 1
                        wrep = []
                        for ci, wv in enumerate(wvs):
                            rhs = wv[:, :, 16 * m:16 * m + 16]
                            pwA = psW.tile([128, 512], f32, tag="wpsA",
                                           name=f"wpsA_{m}_{k}_{ci}")
                            pwB = psW.tile([128, 128], f32, tag="wpsB",
                                           name=f"wpsB_{m}_{k}_{ci}")
                            pAv = pwA.rearrange("p (s x) -> p s x", x=16)
                            for c in range(3):
                                nc.tensor.matmul(
                                    pAv[:, 10 * c:10 * c + 10, :],
                                    sel[:, (4 * k + c) * 128:(4 * k + c + 1) * 128],
                                    rhs, start=True, stop=True)
                            sel3 = sel[:, (4 * k + 3) * 128:(4 * k + 4) * 128]
                            nc.tensor.matmul(pAv[:, 30:32, :], sel3,
                                             rhs[:, 0:2, :], start=True, stop=True)
                            nc.tensor.matmul(
                                pwB.rearrange("p (s x) -> p s x", x=16),
                                sel3, rhs[:, 2:10, :], start=True, stop=True)
                            wsb = gp.tile([128, MCH], bf16, tag=f"wsb{ci}",
                                          name=f"wsb_{m}_{k}_{ci}", bufs=2)
                            nc.scalar.copy(wsb[:, 0:512], pwA)
                            nc.scalar.copy(wsb[:, 512:640], pwB)
                            wrep.append(wsb)
                        s0 = gp.tile([128, 2, MCH], bf16, tag="s0")
                        s1 = gp.tile([128, 2, MCH], bf16, tag="s1")

                        def bc(wsb):
                            return wsb.unsqueeze(1).broadcast_to([128, 2, MCH])

                        nc.vector.tensor_mul(s0, gt[:, 0:2, :], bc(wrep[0]))
                        nc.vector.tensor_mul(s1, gt[:, 2:4, :], bc(wrep[1]))
                        nc.vector.tensor_add(s0, s0, s1)
                        nc.vector.tensor_mul(s1, gb[:, 0:2, :], bc(wrep[2]))
                        nc.vector.tensor_add(s0, s0, s1)
                        nc.vector.tensor_mul(s1, gb[:, 2:4, :], bc(wrep[3]))
                        nc.vector.tensor_add(s0, s0, s1)
                        if dbg and m == 3 and k == 0:
                            for ci in range(4):
                                nc.sync.dma_start(
                                    out=dbg_wsb[:, 640 * ci:640 * (ci + 1)],
                                    in_=wrep[ci][:, :])
                            nc.sync.dma_start(
                                out=dbg_s0[:, :].rearrange(
                                    "p (c n) -> p c n", n=MCH),
                                in_=s0[:, :, :])
                        for mt in range(2):
                            for ct in range(2):
                                st = (k == 0 and ct == 0)
                                sp = (k == 8 and ct == 1)
                                w2s = w2T[:, k, ct, ts(mt, 128)]
                                nc.tensor.matmul(accA[mt], w2s, s0[:, ct, 0:512],
                                                 start=st, stop=sp)
                                nc.tensor.matmul(accB[mt], w2s, s0[:, ct, 512:640],
                                                 start=st, stop=sp)
                    for mt in range(2):
                        nc.scalar.activation(
                            o2v[:, mt, 0:32, 16 * m:16 * m + 16],
                            accA[mt].rearrange("p (s x) -> p s x", x=16),
                            AF.Relu, bias=b2[:, mt:mt + 1])
                        nc.scalar.activation(
                            o2v[:, mt, 32:40, 16 * m:16 * m + 16],
                            accB[mt].rearrange("p (s x) -> p s x", x=16),
                            AF.Relu, bias=b2[:, mt:mt + 1])

                if dbg:
                    nc.sync.dma_start(
                        out=dbg_idx_t[:, :].rearrange(
                            "p (m k t) -> p m k t", k=9, t=48),
                        in_=idxall_t[:, :, :, :])
                    nc.sync.dma_start(
                        out=dbg_out2[:, :].rearrange("p (c n) -> p c n", n=NPOS),
                        in_=out2[:, :, :])

                # ---- Stage D ----
                w3T = cp.tile([128, 2, COUT], bf16)
                for ct in range(2):
                    nc.sync.dma_start(out=w3T[:, ct, :], in_=w3T_d[ct, :, :])
                for ch in range(NCHUNKS):
                    n0 = ch * NCHUNK
                    nn = min(NCHUNK, NVALID - n0)
                    if nn <= 0:
                        break
                    for mt in range(8):
                        xr = iop.tile([128, NCHUNK], bf16, tag="xres")
                        nc.sync.dma_start(
                            out=xr[:, :nn],
                            in_=xs_d[mt, :, PAD * W + n0:PAD * W + n0 + nn])
                        ps = psC.tile([128, NCHUNK], f32, tag=f"daccA{mt % 2}",
                                      name=f"c3ps_{ch}_{mt}")
                        for ct in range(2):
                            nc.tensor.matmul(ps[:, :nn], w3T[:, ct, ts(mt, 128)],
                                             out2[:, ct, n0:n0 + nn],
                                             start=(ct == 0), stop=(ct == 1))
                        t = iop.tile([128, NCHUNK], f32, tag="resid")
                        nc.vector.tensor_add(t[:, :nn], ps[:, :nn], xr[:, :nn])
                        o = iop.tile([128, NCHUNK], bf16, tag="obuf")
                        nc.scalar.activation(o[:, :nn], t[:, :nn], AF.Relu,
                                             bias=b3[:, mt:mt + 1])
                        nc.scalar.dma_start(out=out_d[mt, :, n0:n0 + nn],
                                            in_=o[:, :nn])

    nc.finalize()
    return nc


_NC_CACHE = None


def _get_nc():
    global _NC_CACHE
    if _NC_CACHE is None:
        _NC_CACHE = _build_program()
    return _NC_CACHE


def _prep_inputs(x, w1, s1, b1, w_off, b_off, w2, s2, b2, w3, s3, b3):
    bf16 = ml_dtypes.bfloat16
    f32 = np.float32
    x = np.asarray(x, f32)
    w1f = np.asarray(w1, f32) * np.asarray(s1, f32)[:, None]
    w1T = np.ascontiguousarray(w1f.T.reshape(8, 128, CB)).astype(bf16)
    w1b = np.ascontiguousarray(np.asarray(b1, f32)[None, :]).astype(bf16)
    woffT = np.zeros((9, 2, 128, KOFF), bf16)
    w_off = np.asarray(w_off, f32)
    for tap in range(9):
        ti, tj = divmod(tap, 3)
        wt = w_off[:, :, ti, tj]
        for ct in range(2):
            woffT[tap, ct] = wt[:, ct * 128:(ct + 1) * 128].T.astype(bf16)
    boff = np.asarray(b_off, f32)[:, None]
    w2f = np.asarray(w2, f32) * np.asarray(s2, f32)[:, None, None, None]
    w2T = np.zeros((9, 2, 128, CB), bf16)
    for k in range(9):
        ki, kj = divmod(k, 3)
        wk = w2f[:, :, ki, kj]
        for ct in range(2):
            w2T[k, ct] = wk[:, ct * 128:(ct + 1) * 128].T.astype(bf16)
    b2t = np.ascontiguousarray(np.asarray(b2, f32).reshape(2, 128).T)
    w3f = np.asarray(w3, f32) * np.asarray(s3, f32)[:, None]
    w3T = np.zeros((2, 128, COUT), bf16)
    for ct in range(2):
        w3T[ct] = w3f[:, ct * 128:(ct + 1) * 128].T.astype(bf16)
    b3t = np.ascontiguousarray(np.asarray(b3, f32).reshape(8, 128).T)

    j = np.arange(NPOS)
    y_loc = np.where(j < NVALID, PAD + j // W, 20).astype(f32)
    x_pad = np.where(j < NVALID, PAD + j % W, 50).astype(f32)
    basey = np.zeros((36, NW), f32)
    basex = np.zeros((36, NW), f32)
    kia = np.zeros((36, 1), f32)
    kja = np.zeros((36, 1), f32)
    for k in range(9):
        ki, kj = divmod(k, 3)
        for c in range(4):
            basey[4 * k + c] = y_loc[c * NW:(c + 1) * NW]
            basex[4 * k + c] = x_pad[c * NW:(c + 1) * NW]
            kia[4 * k + c] = ki - 1 + 7.5
            kja[4 * k + c] = kj - 1 + 7.5
    selm = np.zeros((36, 36 * 128), bf16)
    for r in range(36):
        selm[r, r * 128:(r + 1) * 128] = 1.0

    shared = dict(w1T=w1T, w1b=w1b, woffT=np.asarray(woffT), boff=boff,
                  w2T=np.asarray(w2T), b2=b2t, w3T=np.asarray(w3T), b3=b3t,
                  basey=basey, basex=basex, kia=kia, kja=kja,
                  sel=np.asarray(selm))

    in_maps = []
    for core in range(8):
        b, half = core // 2, core % 2
        lo = half * 50
        xs = np.zeros((CIN, RSTRIP, W), f32)
        vlo = max(0, lo - PAD)
        vhi = min(H - 1, lo + 49 + PAD)
        loc0 = vlo - (lo - PAD)
        nrows = vhi - vlo + 1
        xs[:, loc0:loc0 + nrows, :] = x[b, :, vlo:vhi + 1, :]
        indv = np.zeros((RSTRIP, W), f32)
        indv[loc0:loc0 + nrows, :] = 1.0
        vbm = np.zeros((36, 4), f32)
        vbm[:, 0] = loc0 + 8
        vbm[:, 1] = loc0 + nrows - 1 + 8
        vbm[:, 2] = loc0 + 8 - 1
        vbm[:, 3] = loc0 + nrows - 1 + 8 - 1
        in_maps.append(dict(shared,
                            xs=np.ascontiguousarray(
                                xs.reshape(8, 128, RSTRIP * W)).astype(bf16),
                            ind=indv.reshape(1, -1).astype(bf16), vb=vbm))
    return in_maps


def kernel(**inputs):
    from concourse.bass_utils import run_bass_kernel_spmd
    nc = _get_nc()
    in_maps = _prep_inputs(**inputs)
    run_bass_kernel_spmd(nc, in_maps, core_ids=list(range(8)))  # warm-up
    res = run_bass_kernel_spmd(nc, in_maps, core_ids=list(range(8)))
    out = np.zeros((B, COUT, H, W), np.float32)
    for core in range(8):
        b, half = core // 2, core % 2
        lo = half * 50
        o = res.results[core]["out"].reshape(COUT, 50, W)
        out[b, :, lo:lo + 50, :] = np.asarray(o, np.float32)
    return out

